# revision 23
# baseline (speedup 1.0000x reference)
"""Multi-head attention (B=8, N=1024, C=1024, H=16, D=64) on 8 TRN2 NeuronCores.

Strategy: pure data parallelism — one batch element per core, weights
replicated, no collectives.  Per-core dataflow (all layouts chosen so every
matmul contracts over the partition axis):

  x [N,C]  --PE transpose-->  xT [C,N]
  qT = Wq^T@. : lhsT=Wq tile,  rhs=xT      -> [C,N]   (float32r matmuls)
  kT =                        same          -> [C,N]
  v  = x@Wv  : lhsT=xT tile,   rhs=Wv      -> [N,C]   -> v'' bf16 [N, 16*(D+1)]
               (per head: 64 v columns + a ones column for the softmax denom)
  per head h (paired 2 per channel-tile, PE row-tiling K=64):
    scores^T[s,n] = kT_h^T @ qT_h          (K=64)
    p^T = exp(scores^T * scale + mask_bias[s])   (ScalarE, bias kills masked
                                                  KEY rows exactly -> 0)
    o^T[0:64] , denom[64] = v''_h^T @ p^T  (K=1024, m=65, bf16)
    ao^T_h = o^T * (1/denom broadcast)     (DMA partition-broadcast of recip)
  y = ao@Wo + bo : lhsT=aoT tile, rhs=Wo   -> [N,C]

Biases are applied: bq/bk as per-partition adds on the qT/kT copies, bv/bo as
rank-1 (ones x bias) matmul accumulations into PSUM.
"""
import numpy as np

import concourse.bass as bass
import concourse.mybir as mybir
import concourse.tile as tile
from concourse import bacc
from concourse import bass_utils
from concourse.masks import make_identity

f32 = mybir.dt.float32
f32r = mybir.dt.float32r
bf16 = mybir.dt.bfloat16
i32 = mybir.dt.int32

B, N, C, H, D = 8, 1024, 1024, 16, 64
NT = N // 128          # seq tiles
CT = C // 128          # channel tiles
HD = D + 1             # head slice width in v'' (64 v cols + ones col)
SCALE = float(D) ** -0.5
NEG = 30000.0          # exp(-30000) == 0.0 exactly in fp32


def _build():
    nc = bacc.Bacc("TRN2", target_bir_lowering=False, debug=False)

    x_d = nc.declare_dram_parameter("x", [N, C], f32, isOutput=False)
    m_d = nc.declare_dram_parameter("mask", [N], i32, isOutput=False)
    wq_d = nc.declare_dram_parameter("Wq", [C, C], f32, isOutput=False)
    wk_d = nc.declare_dram_parameter("Wk", [C, C], f32, isOutput=False)
    wv_d = nc.declare_dram_parameter("Wv", [C, C], f32, isOutput=False)
    wo_d = nc.declare_dram_parameter("Wo", [C, C], f32, isOutput=False)
    bq_d = nc.declare_dram_parameter("bq", [C], f32, isOutput=False)
    bk_d = nc.declare_dram_parameter("bk", [C], f32, isOutput=False)
    bv_d = nc.declare_dram_parameter("bv", [C], f32, isOutput=False)
    bo_d = nc.declare_dram_parameter("bo", [C], f32, isOutput=False)
    out_d = nc.declare_dram_parameter("out", [N, C], f32, isOutput=True)

    from contextlib import ExitStack
    with ExitStack() as ctx:
        tc = ctx.enter_context(tile.TileContext(nc))
        const = ctx.enter_context(tc.tile_pool(name="const", bufs=1))
        xnp = ctx.enter_context(tc.tile_pool(name="xn", bufs=4))
        xtp = ctx.enter_context(tc.tile_pool(name="xT", bufs=CT))
        qkp = ctx.enter_context(tc.tile_pool(name="qkT", bufs=4))
        v2p = ctx.enter_context(tc.tile_pool(name="v2", bufs=NT))
        ptp = ctx.enter_context(tc.tile_pool(name="pT", bufs=2))
        aop = ctx.enter_context(tc.tile_pool(name="aoT", bufs=CT))
        wqkp = ctx.enter_context(tc.tile_pool(name="wqk", bufs=4))
        whp = ctx.enter_context(tc.tile_pool(name="whalf", bufs=2))
        yp = ctx.enter_context(tc.tile_pool(name="ysb", bufs=2))
        rbp = ctx.enter_context(tc.tile_pool(name="rbc", bufs=1))
        aop65 = ctx.enter_context(tc.tile_pool(name="ao65", bufs=4))
        rcolp = ctx.enter_context(tc.tile_pool(name="rcol", bufs=4))
        rdp = ctx.enter_context(tc.tile_pool(name="rdram", bufs=8, space="DRAM"))
        projps = ctx.enter_context(tc.tile_pool(name="projps", bufs=2, space="PSUM"))
        spool = ctx.enter_context(tc.tile_pool(name="spool", bufs=2, space="PSUM"))
        avps = ctx.enter_context(tc.tile_pool(name="avps", bufs=2, space="PSUM"))

        # ---------------- constants ----------------
        ident = const.tile([128, 128], f32)
        make_identity(nc, ident)

        ones_f = const.tile([1, 128], f32)
        nc.vector.memset(ones_f, 1.0)
        ones_col = ones_f.bitcast(f32r)          # 1.0 is exact in f32r
        ones16 = const.tile([128, H], f32)
        nc.vector.memset(ones16, 1.0)
        ones11 = const.tile([1, 1], f32)
        nc.vector.memset(ones11, 1.0)
        ones_bf = const.tile([1, 128], bf16)
        nc.vector.tensor_copy(ones_bf[:], ones_f[:])
        bo_bf = const.tile([1, C], bf16)

        # mask bias columns: [128, NT]  (partition p, col st) = (mask-1)*NEG
        m_t = const.tile([128, NT], i32)
        nc.sync.dma_start(out=m_t, in_=m_d.ap().rearrange("(t p) -> p t", p=128))
        mb = const.tile([128, NT], f32)
        nc.vector.tensor_scalar(mb[:], m_t[:], -1.0, NEG,
                                op0=mybir.AluOpType.add, op1=mybir.AluOpType.mult)

        # per-partition bias columns for q/k: [128, CT] col ct = bias[ct*128+p]
        bq_t = const.tile([128, CT], f32)
        nc.sync.dma_start(out=bq_t, in_=bq_d.ap().rearrange("(t p) -> p t", p=128))
        bk_t = const.tile([128, CT], f32)
        nc.sync.dma_start(out=bk_t, in_=bk_d.ap().rearrange("(t p) -> p t", p=128))
        # bias rows for v/o rank-1 accumulation
        bv_t = const.tile([1, C], f32r)
        nc.sync.dma_start(out=bv_t, in_=bv_d.ap().bitcast(f32r))
        bo_t = const.tile([1, C], f32)
        nc.sync.dma_start(out=bo_t, in_=bo_d.ap())
        nc.vector.tensor_copy(bo_bf[:], bo_t[:])

        # ---------------- phase 0: load x, transpose to xT ----------------
        xT = []
        for ct in range(CT):
            xT.append(xtp.tile([128, N], f32r, tag="xT", name=f"xT{ct}"))
        xn = []
        for t in range(NT):
            xt_ = xnp.tile([128, C], f32, tag="xn")
            nc.sync.dma_start(out=xt_, in_=x_d.ap()[t * 128:(t + 1) * 128, :])
            xn.append(xt_)
        for j in range(2):           # two groups of 4 seq tiles
            for ct in range(CT):
                trp = projps.tile([128, 512], f32, tag="proj")
                for k in range(4):
                    t = 4 * j + k
                    nc.tensor.transpose(trp[:, k * 128:(k + 1) * 128],
                                        xn[t][:, ct * 128:(ct + 1) * 128],
                                        ident[:])
                nc.vector.tensor_copy(xT[ct][:, j * 512:(j + 1) * 512], trp[:])

        # ---------------- phase a: V projection -> v'' (bf16) ----------------
        v2 = []
        for nt in range(NT):
            v2.append(v2p.tile([128, H, HD], bf16, tag="v2", name=f"v2_{nt}"))
        for qtr in range(4):
            wv_t = whp.tile([128, CT, 256], f32r, tag="whalf")
            nc.scalar.dma_start(
                out=wv_t,
                in_=wv_d.ap().bitcast(f32r).rearrange("(kt p) c -> p kt c", p=128)
                [:, :, qtr * 256:(qtr + 1) * 256])
            for nt in range(NT):
                pv = projps.tile([128, 256], f32, tag="proj")
                nc.tensor.matmul(pv[:], ones_col,
                                 bv_t[:, qtr * 256:(qtr + 1) * 256],
                                 start=True, stop=False)
                for kt in range(CT):
                    nc.tensor.matmul(pv[:], xT[kt][:, nt * 128:(nt + 1) * 128],
                                     wv_t[:, kt, :],
                                     start=False, stop=(kt == CT - 1))
                nc.vector.tensor_copy(
                    v2[nt][:, qtr * 4:(qtr + 1) * 4, 0:D],
                    pv[:].rearrange("p (h d) -> p h d", d=D))
        for nt in range(NT):
            nc.vector.tensor_copy(
                v2[nt][:, :, D:HD],
                ones16.rearrange("p (h one) -> p h one", one=1))

        # prefetch the first half of Wo now (gpsimd cast-DMA f32->bf16) so
        # it is resident long before the output projection needs it
        wo_ts = {}
        for qtr in range(2):
            wo_ts[qtr] = whp.tile([128, CT, 256], bf16, tag="whalf",
                                  name=f"wo{qtr}")
            nc.gpsimd.dma_start(
                out=wo_ts[qtr],
                in_=wo_d.ap().rearrange("(kt p) c -> p kt c", p=128)
                [:, :, qtr * 256:(qtr + 1) * 256])

        # ---------------- phase b: per channel-tile: q/k proj + attention ----
        aoT = []
        for ct in range(CT):
            aoT.append(aop.tile([128, N], bf16, tag="aoT", name=f"aoT{ct}"))

        def recip_normalize(ct, hh, ao65s):
            # denominator row (row 64 of each ao65 half) -> psum COLUMNS via
            # eight K=1 matmuls so the reciprocal runs partition-parallel;
            # then one store + one partition-broadcast DMA (SP queue, kept
            # clear of the bulky weight loads which use the ACT queue).
            dcol = projps.tile([128, 8], f32, tag="proj", name=f"dc{ct}_{hh}")
            for e in range(8):
                nc.tensor.matmul(
                    dcol[:, e:e + 1],
                    ao65s[e // 4][64:65, (e % 4) * 128:(e % 4 + 1) * 128],
                    ones16[64:65, 0:1],
                    start=True, stop=True)
            rcol = rcolp.tile([128, 8], f32, tag="rcol", name=f"rc{ct}_{hh}")
            nc.vector.reciprocal(rcol[:], dcol[:])
            r_dram = rdp.tile([1, N], f32, tag="rdram", name=f"rd{ct}_{hh}")
            nc.sync.dma_start(
                out=r_dram[0, :].rearrange("(e p) -> p e", p=128),
                in_=rcol[:])
            r_bc = rbp.tile([64, N], f32, tag="rbc")
            nc.sync.dma_start(out=r_bc[:],
                              in_=r_dram[0:1, :].partition_broadcast(64))
            for half in range(2):
                nc.vector.tensor_mul(
                    aoT[ct][hh * 64:hh * 64 + 64,
                            half * 512:(half + 1) * 512],
                    ao65s[half][0:64, :],
                    r_bc[:, half * 512:(half + 1) * 512])

        def qk_dma(q):
            # one quarter (256 cols = 2 channel tiles) of Wq/Wk, 1KB bursts
            wq_t = wqkp.tile([128, CT, 256], f32r, tag="wqk", name=f"wq{q}")
            nc.scalar.dma_start(
                out=wq_t,
                in_=wq_d.ap().bitcast(f32r).rearrange("(kt p) c -> p kt c", p=128)
                [:, :, q * 256:(q + 1) * 256])
            wk_t = wqkp.tile([128, CT, 256], f32r, tag="wqk", name=f"wk{q}")
            nc.scalar.dma_start(
                out=wk_t,
                in_=wk_d.ap().bitcast(f32r).rearrange("(kt p) c -> p kt c", p=128)
                [:, :, q * 256:(q + 1) * 256])
            return wq_t, wk_t

        def qk_proj_ops(ct, wq_t, wk_t):
            """Return (qT, kT, ops): ops are deferred closures, executed in
            order, that emit the projection matmuls + copies one at a time so
            they can be interleaved into the scores/exp loop of the previous
            channel tile (keeps the PE busy while ScalarE runs exp)."""
            qT = qkp.tile([128, N], f32r, tag="qkT", name=f"qT{ct}")
            kT = qkp.tile([128, N], f32r, tag="qkT", name=f"kT{ct}")
            ops = []
            state = {}
            for half in range(2):
                for w_t, b_col, dst in ((wq_t, bq_t, qT), (wk_t, bk_t, kT)):
                    def mk_group(half=half, w_t=w_t, b_col=b_col, dst=dst):
                        def alloc():
                            state[(id(w_t), half)] = projps.tile(
                                [128, 512], f32, tag="proj", name="pqk")
                        return alloc
                    alloc = mk_group()
                    c0 = (ct % 2) * 128
                    for kt in range(CT):
                        def mm(kt=kt, half=half, w_t=w_t, alloc=alloc, c0=c0):
                            if kt == 0:
                                alloc()
                            p = state[(id(w_t), half)]
                            nc.tensor.matmul(
                                p[:], w_t[:, kt, c0:c0 + 128],
                                xT[kt][:, half * 512:(half + 1) * 512],
                                start=(kt == 0), stop=(kt == CT - 1))
                        ops.append(mm)
                    def cp(half=half, w_t=w_t, b_col=b_col, dst=dst):
                        p = state[(id(w_t), half)]
                        nc.vector.tensor_scalar_add(
                            dst[:, half * 512:(half + 1) * 512], p[:],
                            b_col[:, ct:ct + 1])
                    ops.append(cp)
            return qT, kT, ops

        wq_quarters = {0: qk_dma(0)}
        qT0, kT0, ops0 = qk_proj_ops(0, *wq_quarters[0])
        for op in ops0:
            op()
        qk_cur = (qT0, kT0)
        next_ops = []
        for ct in range(CT):
            qT, kT = qk_cur
            # prefetch the weight quarter two channel-tiles ahead
            nq = (ct + 2) // 2
            if ct % 2 == 0 and ct + 2 < CT and nq not in wq_quarters:
                wq_quarters[nq] = qk_dma(nq)
            if ct + 1 < CT:
                qTn, kTn, next_ops = qk_proj_ops(ct + 1,
                                                 *wq_quarters[(ct + 1) // 2])
            else:
                qTn = kTn = None
                next_ops = []
            # scores + exp for the 2 heads of this ct, st-wise; AV half-0
            # accumulation chunks trail the exp by one seq tile so the PE
            # never waits on ScalarE.
            pts = []
            for hh in range(2):
                pt = ptp.tile([128, NT, N], bf16, tag="pT", name=f"pT{ct}_{hh}")
                pts.append(pt)
            av0 = []
            for hh in range(2):
                av0.append(avps.tile([65, 512], f32, tag="av",
                                     name=f"av0_{ct}_{hh}"))

            def av0_chunk(st):
                for hh in range(2):
                    nc.tensor.matmul(
                        av0[hh][:],
                        v2[st][:, 2 * ct + hh, :],
                        pts[hh][:, st, 0:512],
                        start=(st == 0), stop=(st == NT - 1))

            for st in range(NT):
                for hh in range(2):
                    r0, r1 = hh * 64, hh * 64 + 64
                    ps = spool.tile([128, N], f32, tag="scores")
                    for half in range(2):
                        nc.tensor.matmul(
                            ps[:, half * 512:(half + 1) * 512],
                            kT[r0:r1, st * 128:(st + 1) * 128],
                            qT[r0:r1, half * 512:(half + 1) * 512],
                            start=True, stop=True)
                    nc.scalar.activation(out=pts[hh][:, st, :], in_=ps[:],
                                         func=mybir.ActivationFunctionType.Exp,
                                         bias=mb[:, st:st + 1], scale=SCALE)
                if st > 1:
                    av0_chunk(st - 2)   # 2 tiles behind: exp surely drained
                # interleave ~5 of the next ct's projection ops to keep the
                # PE fed while ScalarE churns through the exps
                for _ in range(5):
                    if next_ops:
                        next_ops.pop(0)()
            av0_chunk(NT - 2)
            while next_ops:
                next_ops.pop(0)()
            av0_chunk(NT - 1)
            if ct + 1 < CT:
                qk_cur = (qTn, kTn)

            ao65s = {}
            for hh in range(2):
                t = aop65.tile([65, 512], f32, tag="ao65",
                               name=f"ao65_{ct}_{hh}_0")
                nc.vector.tensor_copy(t[:], av0[hh][:])   # frees the bank
                ao65s[hh] = [t]
            # AV half-1: contiguous PE block (exp for this ct already done)
            for hh in range(2):
                av1 = avps.tile([65, 512], f32, tag="av",
                                name=f"av1_{ct}_{hh}")
                for st in range(NT):
                    nc.tensor.matmul(
                        av1[:],
                        v2[st][:, 2 * ct + hh, :],
                        pts[hh][:, st, 512:1024],
                        start=(st == 0), stop=(st == NT - 1))
                t = aop65.tile([65, 512], f32, tag="ao65",
                               name=f"ao65_{ct}_{hh}_1")
                nc.vector.tensor_copy(t[:], av1[:])
                ao65s[hh].append(t)
            for hh in range(2):
                recip_normalize(ct, hh, ao65s[hh])

        # ---------------- phase c: output projection ----------------
        for qtr in range(4):
            if qtr in wo_ts:
                wo_t = wo_ts[qtr]
            else:
                wo_t = whp.tile([128, CT, 256], bf16, tag="whalf",
                                name=f"wo{qtr}")
                nc.gpsimd.dma_start(
                    out=wo_t,
                    in_=wo_d.ap().rearrange(
                        "(kt p) c -> p kt c", p=128)
                    [:, :, qtr * 256:(qtr + 1) * 256])
            for nt in range(NT):
                py = projps.tile([128, 256], f32, tag="proj")
                nc.tensor.matmul(py[:], ones_bf[:],
                                 bo_bf[:, qtr * 256:(qtr + 1) * 256],
                                 start=True, stop=False)
                for kt in range(CT):
                    nc.tensor.matmul(py[:], aoT[kt][:, nt * 128:(nt + 1) * 128],
                                     wo_t[:, kt, :],
                                     start=False, stop=(kt == CT - 1))
                y = yp.tile([128, 256], f32, tag="ysb")
                nc.vector.tensor_copy(y[:], py[:])
                nc.sync.dma_start(
                    out=out_d.ap()[nt * 128:(nt + 1) * 128,
                                   qtr * 256:(qtr + 1) * 256],
                    in_=y[:])

    nc.compile()
    return nc


_NC = None


def _get_nc():
    global _NC
    if _NC is None:
        _NC = _build()
    return _NC


def _in_maps(inputs):
    q = np.ascontiguousarray(np.asarray(inputs["query"], dtype=np.float32))
    mask = np.ascontiguousarray(np.asarray(inputs["mask"], dtype=np.int32))
    shared = {
        "Wq": np.ascontiguousarray(np.asarray(inputs["Wq"], dtype=np.float32)),
        "Wk": np.ascontiguousarray(np.asarray(inputs["Wk"], dtype=np.float32)),
        "Wv": np.ascontiguousarray(np.asarray(inputs["Wv"], dtype=np.float32)),
        "Wo": np.ascontiguousarray(np.asarray(inputs["Wo"], dtype=np.float32)),
        "bq": np.ascontiguousarray(np.asarray(inputs["bq"], dtype=np.float32)),
        "bk": np.ascontiguousarray(np.asarray(inputs["bk"], dtype=np.float32)),
        "bv": np.ascontiguousarray(np.asarray(inputs["bv"], dtype=np.float32)),
        "bo": np.ascontiguousarray(np.asarray(inputs["bo"], dtype=np.float32)),
    }
    in_maps = []
    for b in range(B):
        m = {"x": np.ascontiguousarray(q[b]), "mask": np.ascontiguousarray(mask[b])}
        m.update(shared)
        in_maps.append(m)
    return in_maps


def kernel(**inputs):
    nc = _get_nc()
    res = bass_utils.run_bass_kernel_spmd(nc, _in_maps(inputs),
                                          core_ids=list(range(B)))
    return np.stack([r["out"] for r in res.results]).astype(np.float32)


if __name__ == "__main__":
    rng = np.random.default_rng(0)
    inputs = {
        "query": rng.standard_normal((B, N, C), dtype=np.float32),
        "mask": rng.integers(0, 2, (B, N)).astype(np.int32),
        "Wq": (rng.standard_normal((C, C), dtype=np.float32) * C ** -0.5),
        "bq": np.zeros(C, np.float32),
        "Wk": (rng.standard_normal((C, C), dtype=np.float32) * C ** -0.5),
        "bk": np.zeros(C, np.float32),
        "Wv": (rng.standard_normal((C, C), dtype=np.float32) * C ** -0.5),
        "bv": np.zeros(C, np.float32),
        "Wo": (rng.standard_normal((C, C), dtype=np.float32) * C ** -0.5),
        "bo": np.zeros(C, np.float32),
    }
    out = kernel(**inputs)
    # numpy reference
    def ref(q, mask, Wq, bq, Wk, bk, Wv, bv, Wo, bo):
        Bq, Nq, Cq = q.shape
        qq = (q @ Wq + bq).reshape(Bq, Nq, H, D).transpose(0, 2, 1, 3)
        kk = (q @ Wk + bk).reshape(Bq, Nq, H, D).transpose(0, 2, 1, 3)
        vv = (q @ Wv + bv).reshape(Bq, Nq, H, D).transpose(0, 2, 1, 3)
        at = np.einsum("bhnd,bhsd->bhns", qq, kk) * SCALE
        at = np.where(mask[:, None, None, :] == 0, -np.inf, at)
        at = at - at.max(-1, keepdims=True)
        e = np.exp(at)
        p = e / e.sum(-1, keepdims=True)
        o = np.einsum("bhns,bhsd->bhnd", p, vv)
        o = o.transpose(0, 2, 1, 3).reshape(Bq, Nq, Cq)
        return o @ Wo + bo
    expected = ref(**{k: inputs[k] for k in
                      ["query", "mask", "Wq", "bq", "Wk", "bk", "Wv", "bv", "Wo", "bo"]}
                   .values()) if False else None
    expected = ref(inputs["query"], inputs["mask"], inputs["Wq"], inputs["bq"],
                   inputs["Wk"], inputs["bk"], inputs["Wv"], inputs["bv"],
                   inputs["Wo"], inputs["bo"])
    err = np.abs(out - expected).max() / np.abs(expected).max()
    print("self-test rel err:", err)


# revision 24
# speedup vs baseline: 1.0169x; 1.0169x over previous
"""Multi-head attention (B=8, N=1024, C=1024, H=16, D=64) on 8 TRN2 NeuronCores.

Strategy: pure data parallelism — one batch element per core, weights
replicated, no collectives.  Per-core dataflow (all layouts chosen so every
matmul contracts over the partition axis):

  x [N,C]  --PE transpose-->  xT [C,N]
  qT = Wq^T@. : lhsT=Wq tile,  rhs=xT      -> [C,N]   (float32r matmuls)
  kT =                        same          -> [C,N]
  v  = x@Wv  : lhsT=xT tile,   rhs=Wv      -> [N,C]   -> v'' bf16 [N, 16*(D+1)]
               (per head: 64 v columns + a ones column for the softmax denom)
  per head h (paired 2 per channel-tile, PE row-tiling K=64):
    scores^T[s,n] = kT_h^T @ qT_h          (K=64)
    p^T = exp(scores^T * scale + mask_bias[s])   (ScalarE, bias kills masked
                                                  KEY rows exactly -> 0)
    o^T[0:64] , denom[64] = v''_h^T @ p^T  (K=1024, m=65, bf16)
    ao^T_h = o^T * (1/denom broadcast)     (DMA partition-broadcast of recip)
  y = ao@Wo + bo : lhsT=aoT tile, rhs=Wo   -> [N,C]

Biases are applied: bq/bk as per-partition adds on the qT/kT copies, bv/bo as
rank-1 (ones x bias) matmul accumulations into PSUM.
"""
import numpy as np

import concourse.bass as bass
import concourse.mybir as mybir
import concourse.tile as tile
from concourse import bacc
from concourse import bass_utils
from concourse.masks import make_identity

f32 = mybir.dt.float32
f32r = mybir.dt.float32r
bf16 = mybir.dt.bfloat16
i32 = mybir.dt.int32

B, N, C, H, D = 8, 1024, 1024, 16, 64
NT = N // 128          # seq tiles
CT = C // 128          # channel tiles
HD = D + 1             # head slice width in v'' (64 v cols + ones col)
SCALE = float(D) ** -0.5
NEG = 30000.0          # exp(-30000) == 0.0 exactly in fp32


def _build():
    nc = bacc.Bacc("TRN2", target_bir_lowering=False, debug=False)

    x_d = nc.declare_dram_parameter("x", [N, C], f32, isOutput=False)
    m_d = nc.declare_dram_parameter("mask", [N], i32, isOutput=False)
    wq_d = nc.declare_dram_parameter("Wq", [C, C], f32, isOutput=False)
    wk_d = nc.declare_dram_parameter("Wk", [C, C], f32, isOutput=False)
    wv_d = nc.declare_dram_parameter("Wv", [C, C], f32, isOutput=False)
    wo_d = nc.declare_dram_parameter("Wo", [C, C], f32, isOutput=False)
    bq_d = nc.declare_dram_parameter("bq", [C], f32, isOutput=False)
    bk_d = nc.declare_dram_parameter("bk", [C], f32, isOutput=False)
    bv_d = nc.declare_dram_parameter("bv", [C], f32, isOutput=False)
    bo_d = nc.declare_dram_parameter("bo", [C], f32, isOutput=False)
    out_d = nc.declare_dram_parameter("out", [N, C], f32, isOutput=True)

    from contextlib import ExitStack
    with ExitStack() as ctx:
        tc = ctx.enter_context(tile.TileContext(nc))
        const = ctx.enter_context(tc.tile_pool(name="const", bufs=1))
        xnp = ctx.enter_context(tc.tile_pool(name="xn", bufs=4))
        xtp = ctx.enter_context(tc.tile_pool(name="xT", bufs=CT))
        qkp = ctx.enter_context(tc.tile_pool(name="qkT", bufs=4))
        v2p = ctx.enter_context(tc.tile_pool(name="v2", bufs=NT))
        ptp = ctx.enter_context(tc.tile_pool(name="pT", bufs=2))
        aop = ctx.enter_context(tc.tile_pool(name="aoT", bufs=CT))
        wqkp = ctx.enter_context(tc.tile_pool(name="wqk", bufs=4))
        whp = ctx.enter_context(tc.tile_pool(name="whalf", bufs=2))
        yp = ctx.enter_context(tc.tile_pool(name="ysb", bufs=2))
        rbp = ctx.enter_context(tc.tile_pool(name="rbc", bufs=1))
        aop65 = ctx.enter_context(tc.tile_pool(name="ao65", bufs=4))
        rcolp = ctx.enter_context(tc.tile_pool(name="rcol", bufs=4))
        rdp = ctx.enter_context(tc.tile_pool(name="rdram", bufs=8, space="DRAM"))
        projps = ctx.enter_context(tc.tile_pool(name="projps", bufs=2, space="PSUM"))
        spool = ctx.enter_context(tc.tile_pool(name="spool", bufs=2, space="PSUM"))
        avps = ctx.enter_context(tc.tile_pool(name="avps", bufs=2, space="PSUM"))

        # ---------------- constants ----------------
        ident = const.tile([128, 128], f32)
        make_identity(nc, ident)

        ones_f = const.tile([1, 128], f32)
        nc.vector.memset(ones_f, 1.0)
        ones_col = ones_f.bitcast(f32r)          # 1.0 is exact in f32r
        ones16 = const.tile([128, H], f32)
        nc.vector.memset(ones16, 1.0)
        ones11 = const.tile([1, 1], f32)
        nc.vector.memset(ones11, 1.0)
        ones_bf = const.tile([1, 128], bf16)
        nc.vector.tensor_copy(ones_bf[:], ones_f[:])
        bo_bf = const.tile([1, C], bf16)

        # mask bias columns: [128, NT]  (partition p, col st) = (mask-1)*NEG
        m_t = const.tile([128, NT], i32)
        nc.sync.dma_start(out=m_t, in_=m_d.ap().rearrange("(t p) -> p t", p=128))
        mb = const.tile([128, NT], f32)
        nc.vector.tensor_scalar(mb[:], m_t[:], -1.0, NEG,
                                op0=mybir.AluOpType.add, op1=mybir.AluOpType.mult)

        # per-partition bias columns for q/k: [128, CT] col ct = bias[ct*128+p]
        bq_t = const.tile([128, CT], f32)
        nc.sync.dma_start(out=bq_t, in_=bq_d.ap().rearrange("(t p) -> p t", p=128))
        bk_t = const.tile([128, CT], f32)
        nc.sync.dma_start(out=bk_t, in_=bk_d.ap().rearrange("(t p) -> p t", p=128))
        # bias rows for v/o rank-1 accumulation
        bv_t = const.tile([1, C], f32r)
        nc.sync.dma_start(out=bv_t, in_=bv_d.ap().bitcast(f32r))
        bo_t = const.tile([1, C], f32)
        nc.sync.dma_start(out=bo_t, in_=bo_d.ap())
        nc.vector.tensor_copy(bo_bf[:], bo_t[:])

        # PE warmup: ~4us of dummy matmuls on the identity so the HAM
        # clock-gate reaches 2.4 GHz before the transposes/projections start
        warm_ps = projps.tile([128, 512], f32, tag="proj", name="warm")
        for w in range(24):
            nc.tensor.matmul(warm_ps[:, 0:128], ident[:], ident[:],
                             start=True, stop=True)

        # ---------------- phase 0: load x, transpose to xT ----------------
        xT = []
        for ct in range(CT):
            xT.append(xtp.tile([128, N], f32r, tag="xT", name=f"xT{ct}"))
        xn = []
        for t in range(NT):
            xt_ = xnp.tile([128, C], f32, tag="xn")
            nc.sync.dma_start(out=xt_, in_=x_d.ap()[t * 128:(t + 1) * 128, :])
            xn.append(xt_)
        for j in range(2):           # two groups of 4 seq tiles
            for ct in range(CT):
                trp = projps.tile([128, 512], f32, tag="proj")
                for k in range(4):
                    t = 4 * j + k
                    nc.tensor.transpose(trp[:, k * 128:(k + 1) * 128],
                                        xn[t][:, ct * 128:(ct + 1) * 128],
                                        ident[:])
                nc.vector.tensor_copy(xT[ct][:, j * 512:(j + 1) * 512], trp[:])

        # ---------------- phase a: V projection -> v'' (bf16) ----------------
        v2 = []
        for nt in range(NT):
            v2.append(v2p.tile([128, H, HD], bf16, tag="v2", name=f"v2_{nt}"))
        for qtr in range(4):
            wv_t = whp.tile([128, CT, 256], f32r, tag="whalf")
            nc.scalar.dma_start(
                out=wv_t,
                in_=wv_d.ap().bitcast(f32r).rearrange("(kt p) c -> p kt c", p=128)
                [:, :, qtr * 256:(qtr + 1) * 256])
            for nt in range(NT):
                pv = projps.tile([128, 256], f32, tag="proj")
                nc.tensor.matmul(pv[:], ones_col,
                                 bv_t[:, qtr * 256:(qtr + 1) * 256],
                                 start=True, stop=False)
                for kt in range(CT):
                    nc.tensor.matmul(pv[:], xT[kt][:, nt * 128:(nt + 1) * 128],
                                     wv_t[:, kt, :],
                                     start=False, stop=(kt == CT - 1))
                nc.vector.tensor_copy(
                    v2[nt][:, qtr * 4:(qtr + 1) * 4, 0:D],
                    pv[:].rearrange("p (h d) -> p h d", d=D))
        for nt in range(NT):
            nc.vector.tensor_copy(
                v2[nt][:, :, D:HD],
                ones16.rearrange("p (h one) -> p h one", one=1))

        # prefetch the first half of Wo now (gpsimd cast-DMA f32->bf16) so
        # it is resident long before the output projection needs it
        wo_ts = {}
        for qtr in range(2):
            wo_ts[qtr] = whp.tile([128, CT, 256], bf16, tag="whalf",
                                  name=f"wo{qtr}")
            nc.gpsimd.dma_start(
                out=wo_ts[qtr],
                in_=wo_d.ap().rearrange("(kt p) c -> p kt c", p=128)
                [:, :, qtr * 256:(qtr + 1) * 256])

        # ---------------- phase b: per channel-tile: q/k proj + attention ----
        aoT = []
        for ct in range(CT):
            aoT.append(aop.tile([128, N], bf16, tag="aoT", name=f"aoT{ct}"))

        def recip_normalize(ct, hh, ao65s):
            # denominator row (row 64 of each ao65 half) -> psum COLUMNS via
            # eight K=1 matmuls so the reciprocal runs partition-parallel;
            # then one store + one partition-broadcast DMA (SP queue, kept
            # clear of the bulky weight loads which use the ACT queue).
            dcol = projps.tile([128, 8], f32, tag="proj", name=f"dc{ct}_{hh}")
            for e in range(8):
                nc.tensor.matmul(
                    dcol[:, e:e + 1],
                    ao65s[e // 4][64:65, (e % 4) * 128:(e % 4 + 1) * 128],
                    ones16[64:65, 0:1],
                    start=True, stop=True)
            rcol = rcolp.tile([128, 8], f32, tag="rcol", name=f"rc{ct}_{hh}")
            nc.vector.reciprocal(rcol[:], dcol[:])
            r_dram = rdp.tile([1, N], f32, tag="rdram", name=f"rd{ct}_{hh}")
            nc.sync.dma_start(
                out=r_dram[0, :].rearrange("(e p) -> p e", p=128),
                in_=rcol[:])
            r_bc = rbp.tile([64, N], f32, tag="rbc")
            nc.sync.dma_start(out=r_bc[:],
                              in_=r_dram[0:1, :].partition_broadcast(64))
            for half in range(2):
                nc.vector.tensor_mul(
                    aoT[ct][hh * 64:hh * 64 + 64,
                            half * 512:(half + 1) * 512],
                    ao65s[half][0:64, :],
                    r_bc[:, half * 512:(half + 1) * 512])

        def qk_dma(q):
            # one quarter (256 cols = 2 channel tiles) of Wq/Wk, 1KB bursts
            wq_t = wqkp.tile([128, CT, 256], f32r, tag="wqk", name=f"wq{q}")
            nc.scalar.dma_start(
                out=wq_t,
                in_=wq_d.ap().bitcast(f32r).rearrange("(kt p) c -> p kt c", p=128)
                [:, :, q * 256:(q + 1) * 256])
            wk_t = wqkp.tile([128, CT, 256], f32r, tag="wqk", name=f"wk{q}")
            nc.scalar.dma_start(
                out=wk_t,
                in_=wk_d.ap().bitcast(f32r).rearrange("(kt p) c -> p kt c", p=128)
                [:, :, q * 256:(q + 1) * 256])
            return wq_t, wk_t

        def qk_proj_ops(ct, wq_t, wk_t):
            """Return (qT, kT, ops): ops are deferred closures, executed in
            order, that emit the projection matmuls + copies one at a time so
            they can be interleaved into the scores/exp loop of the previous
            channel tile (keeps the PE busy while ScalarE runs exp)."""
            qT = qkp.tile([128, N], f32r, tag="qkT", name=f"qT{ct}")
            kT = qkp.tile([128, N], f32r, tag="qkT", name=f"kT{ct}")
            ops = []
            state = {}
            for half in range(2):
                for w_t, b_col, dst in ((wq_t, bq_t, qT), (wk_t, bk_t, kT)):
                    def mk_group(half=half, w_t=w_t, b_col=b_col, dst=dst):
                        def alloc():
                            state[(id(w_t), half)] = projps.tile(
                                [128, 512], f32, tag="proj", name="pqk")
                        return alloc
                    alloc = mk_group()
                    c0 = (ct % 2) * 128
                    for kt in range(CT):
                        def mm(kt=kt, half=half, w_t=w_t, alloc=alloc, c0=c0):
                            if kt == 0:
                                alloc()
                            p = state[(id(w_t), half)]
                            nc.tensor.matmul(
                                p[:], w_t[:, kt, c0:c0 + 128],
                                xT[kt][:, half * 512:(half + 1) * 512],
                                start=(kt == 0), stop=(kt == CT - 1))
                        ops.append(mm)
                    def cp(half=half, w_t=w_t, b_col=b_col, dst=dst):
                        p = state[(id(w_t), half)]
                        nc.vector.tensor_scalar_add(
                            dst[:, half * 512:(half + 1) * 512], p[:],
                            b_col[:, ct:ct + 1])
                    ops.append(cp)
            return qT, kT, ops

        wq_quarters = {0: qk_dma(0)}
        qT0, kT0, ops0 = qk_proj_ops(0, *wq_quarters[0])
        for op in ops0:
            op()
        qk_cur = (qT0, kT0)
        next_ops = []
        for ct in range(CT):
            qT, kT = qk_cur
            # prefetch the weight quarter two channel-tiles ahead
            nq = (ct + 2) // 2
            if ct % 2 == 0 and ct + 2 < CT and nq not in wq_quarters:
                wq_quarters[nq] = qk_dma(nq)
            if ct + 1 < CT:
                qTn, kTn, next_ops = qk_proj_ops(ct + 1,
                                                 *wq_quarters[(ct + 1) // 2])
            else:
                qTn = kTn = None
                next_ops = []
            # scores + exp for the 2 heads of this ct, st-wise; AV half-0
            # accumulation chunks trail the exp by one seq tile so the PE
            # never waits on ScalarE.
            pts = []
            for hh in range(2):
                pt = ptp.tile([128, NT, N], bf16, tag="pT", name=f"pT{ct}_{hh}")
                pts.append(pt)
            av0 = []
            for hh in range(2):
                av0.append(avps.tile([65, 512], f32, tag="av",
                                     name=f"av0_{ct}_{hh}"))

            def av0_chunk(st):
                for hh in range(2):
                    nc.tensor.matmul(
                        av0[hh][:],
                        v2[st][:, 2 * ct + hh, :],
                        pts[hh][:, st, 0:512],
                        start=(st == 0), stop=(st == NT - 1))

            for st in range(NT):
                for hh in range(2):
                    r0, r1 = hh * 64, hh * 64 + 64
                    ps = spool.tile([128, N], f32, tag="scores")
                    for half in range(2):
                        nc.tensor.matmul(
                            ps[:, half * 512:(half + 1) * 512],
                            kT[r0:r1, st * 128:(st + 1) * 128],
                            qT[r0:r1, half * 512:(half + 1) * 512],
                            start=True, stop=True)
                    nc.scalar.activation(out=pts[hh][:, st, :], in_=ps[:],
                                         func=mybir.ActivationFunctionType.Exp,
                                         bias=mb[:, st:st + 1], scale=SCALE)
                if st > 1:
                    av0_chunk(st - 2)   # 2 tiles behind: exp surely drained
                # interleave ~5 of the next ct's projection ops to keep the
                # PE fed while ScalarE churns through the exps
                for _ in range(6):
                    if next_ops:
                        next_ops.pop(0)()
            av0_chunk(NT - 2)
            while next_ops:
                next_ops.pop(0)()
            av0_chunk(NT - 1)
            if ct + 1 < CT:
                qk_cur = (qTn, kTn)

            ao65s = {}
            for hh in range(2):
                t = aop65.tile([65, 512], f32, tag="ao65",
                               name=f"ao65_{ct}_{hh}_0")
                nc.vector.tensor_copy(t[:], av0[hh][:])   # frees the bank
                ao65s[hh] = [t]
            # AV half-1: contiguous PE block (exp for this ct already done)
            for hh in range(2):
                av1 = avps.tile([65, 512], f32, tag="av",
                                name=f"av1_{ct}_{hh}")
                for st in range(NT):
                    nc.tensor.matmul(
                        av1[:],
                        v2[st][:, 2 * ct + hh, :],
                        pts[hh][:, st, 512:1024],
                        start=(st == 0), stop=(st == NT - 1))
                t = aop65.tile([65, 512], f32, tag="ao65",
                               name=f"ao65_{ct}_{hh}_1")
                nc.vector.tensor_copy(t[:], av1[:])
                ao65s[hh].append(t)
            for hh in range(2):
                recip_normalize(ct, hh, ao65s[hh])

        # ---------------- phase c: output projection ----------------
        for qtr in range(4):
            if qtr in wo_ts:
                wo_t = wo_ts[qtr]
            else:
                wo_t = whp.tile([128, CT, 256], bf16, tag="whalf",
                                name=f"wo{qtr}")
                nc.gpsimd.dma_start(
                    out=wo_t,
                    in_=wo_d.ap().rearrange(
                        "(kt p) c -> p kt c", p=128)
                    [:, :, qtr * 256:(qtr + 1) * 256])
            for nt in range(NT):
                py = projps.tile([128, 256], f32, tag="proj")
                nc.tensor.matmul(py[:], ones_bf[:],
                                 bo_bf[:, qtr * 256:(qtr + 1) * 256],
                                 start=True, stop=False)
                for kt in range(CT):
                    nc.tensor.matmul(py[:], aoT[kt][:, nt * 128:(nt + 1) * 128],
                                     wo_t[:, kt, :],
                                     start=False, stop=(kt == CT - 1))
                y = yp.tile([128, 256], f32, tag="ysb")
                nc.vector.tensor_copy(y[:], py[:])
                nc.sync.dma_start(
                    out=out_d.ap()[nt * 128:(nt + 1) * 128,
                                   qtr * 256:(qtr + 1) * 256],
                    in_=y[:])

    nc.compile()
    return nc


_NC = None


def _get_nc():
    global _NC
    if _NC is None:
        _NC = _build()
    return _NC


def _in_maps(inputs):
    q = np.ascontiguousarray(np.asarray(inputs["query"], dtype=np.float32))
    mask = np.ascontiguousarray(np.asarray(inputs["mask"], dtype=np.int32))
    shared = {
        "Wq": np.ascontiguousarray(np.asarray(inputs["Wq"], dtype=np.float32)),
        "Wk": np.ascontiguousarray(np.asarray(inputs["Wk"], dtype=np.float32)),
        "Wv": np.ascontiguousarray(np.asarray(inputs["Wv"], dtype=np.float32)),
        "Wo": np.ascontiguousarray(np.asarray(inputs["Wo"], dtype=np.float32)),
        "bq": np.ascontiguousarray(np.asarray(inputs["bq"], dtype=np.float32)),
        "bk": np.ascontiguousarray(np.asarray(inputs["bk"], dtype=np.float32)),
        "bv": np.ascontiguousarray(np.asarray(inputs["bv"], dtype=np.float32)),
        "bo": np.ascontiguousarray(np.asarray(inputs["bo"], dtype=np.float32)),
    }
    in_maps = []
    for b in range(B):
        m = {"x": np.ascontiguousarray(q[b]), "mask": np.ascontiguousarray(mask[b])}
        m.update(shared)
        in_maps.append(m)
    return in_maps


def kernel(**inputs):
    nc = _get_nc()
    res = bass_utils.run_bass_kernel_spmd(nc, _in_maps(inputs),
                                          core_ids=list(range(B)))
    return np.stack([r["out"] for r in res.results]).astype(np.float32)


if __name__ == "__main__":
    rng = np.random.default_rng(0)
    inputs = {
        "query": rng.standard_normal((B, N, C), dtype=np.float32),
        "mask": rng.integers(0, 2, (B, N)).astype(np.int32),
        "Wq": (rng.standard_normal((C, C), dtype=np.float32) * C ** -0.5),
        "bq": np.zeros(C, np.float32),
        "Wk": (rng.standard_normal((C, C), dtype=np.float32) * C ** -0.5),
        "bk": np.zeros(C, np.float32),
        "Wv": (rng.standard_normal((C, C), dtype=np.float32) * C ** -0.5),
        "bv": np.zeros(C, np.float32),
        "Wo": (rng.standard_normal((C, C), dtype=np.float32) * C ** -0.5),
        "bo": np.zeros(C, np.float32),
    }
    out = kernel(**inputs)
    # numpy reference
    def ref(q, mask, Wq, bq, Wk, bk, Wv, bv, Wo, bo):
        Bq, Nq, Cq = q.shape
        qq = (q @ Wq + bq).reshape(Bq, Nq, H, D).transpose(0, 2, 1, 3)
        kk = (q @ Wk + bk).reshape(Bq, Nq, H, D).transpose(0, 2, 1, 3)
        vv = (q @ Wv + bv).reshape(Bq, Nq, H, D).transpose(0, 2, 1, 3)
        at = np.einsum("bhnd,bhsd->bhns", qq, kk) * SCALE
        at = np.where(mask[:, None, None, :] == 0, -np.inf, at)
        at = at - at.max(-1, keepdims=True)
        e = np.exp(at)
        p = e / e.sum(-1, keepdims=True)
        o = np.einsum("bhns,bhsd->bhnd", p, vv)
        o = o.transpose(0, 2, 1, 3).reshape(Bq, Nq, Cq)
        return o @ Wo + bo
    expected = ref(**{k: inputs[k] for k in
                      ["query", "mask", "Wq", "bq", "Wk", "bk", "Wv", "bv", "Wo", "bo"]}
                   .values()) if False else None
    expected = ref(inputs["query"], inputs["mask"], inputs["Wq"], inputs["bq"],
                   inputs["Wk"], inputs["bk"], inputs["Wv"], inputs["bv"],
                   inputs["Wo"], inputs["bo"])
    err = np.abs(out - expected).max() / np.abs(expected).max()
    print("self-test rel err:", err)


# revision 25
# speedup vs baseline: 1.0500x; 1.0326x over previous
"""Multi-head attention (B=8, N=1024, C=1024, H=16, D=64) on 8 TRN2 NeuronCores.

Strategy: pure data parallelism — one batch element per core, weights
replicated, no collectives.  Per-core dataflow (all layouts chosen so every
matmul contracts over the partition axis):

  x [N,C]  --PE transpose-->  xT [C,N]
  qT = Wq^T@. : lhsT=Wq tile,  rhs=xT      -> [C,N]   (float32r matmuls)
  kT =                        same          -> [C,N]
  v  = x@Wv  : lhsT=xT tile,   rhs=Wv      -> [N,C]   -> v'' bf16 [N, 16*(D+1)]
               (per head: 64 v columns + a ones column for the softmax denom)
  per head h (paired 2 per channel-tile, PE row-tiling K=64):
    scores^T[s,n] = kT_h^T @ qT_h          (K=64)
    p^T = exp(scores^T * scale + mask_bias[s])   (ScalarE, bias kills masked
                                                  KEY rows exactly -> 0)
    o^T[0:64] , denom[64] = v''_h^T @ p^T  (K=1024, m=65, bf16)
    ao^T_h = o^T * (1/denom broadcast)     (DMA partition-broadcast of recip)
  y = ao@Wo + bo : lhsT=aoT tile, rhs=Wo   -> [N,C]

Biases are applied: bq/bk as per-partition adds on the qT/kT copies, bv/bo as
rank-1 (ones x bias) matmul accumulations into PSUM.
"""
import numpy as np

import concourse.bass as bass
import concourse.mybir as mybir
import concourse.tile as tile
from concourse import bacc
from concourse import bass_utils
from concourse.masks import make_identity

f32 = mybir.dt.float32
f32r = mybir.dt.float32r
bf16 = mybir.dt.bfloat16
i32 = mybir.dt.int32

B, N, C, H, D = 8, 1024, 1024, 16, 64
NT = N // 128          # seq tiles
CT = C // 128          # channel tiles
HD = D + 1             # head slice width in v'' (64 v cols + ones col)
SCALE = float(D) ** -0.5
NEG = 30000.0          # exp(-30000) == 0.0 exactly in fp32


def _build():
    nc = bacc.Bacc("TRN2", target_bir_lowering=False, debug=False)

    x_d = nc.declare_dram_parameter("x", [N, C], f32, isOutput=False)
    m_d = nc.declare_dram_parameter("mask", [N], i32, isOutput=False)
    wq_d = nc.declare_dram_parameter("Wq", [C, C], f32, isOutput=False)
    wk_d = nc.declare_dram_parameter("Wk", [C, C], f32, isOutput=False)
    wv_d = nc.declare_dram_parameter("Wv", [C, C], f32, isOutput=False)
    wo_d = nc.declare_dram_parameter("Wo", [C, C], f32, isOutput=False)
    bq_d = nc.declare_dram_parameter("bq", [C], f32, isOutput=False)
    bk_d = nc.declare_dram_parameter("bk", [C], f32, isOutput=False)
    bv_d = nc.declare_dram_parameter("bv", [C], f32, isOutput=False)
    bo_d = nc.declare_dram_parameter("bo", [C], f32, isOutput=False)
    out_d = nc.declare_dram_parameter("out", [N, C], f32, isOutput=True)

    from contextlib import ExitStack
    with ExitStack() as ctx:
        tc = ctx.enter_context(tile.TileContext(nc))
        const = ctx.enter_context(tc.tile_pool(name="const", bufs=1))
        xnp = ctx.enter_context(tc.tile_pool(name="xn", bufs=4))
        xtp = ctx.enter_context(tc.tile_pool(name="xT", bufs=CT))
        qkp = ctx.enter_context(tc.tile_pool(name="qkT", bufs=4))
        v2p = ctx.enter_context(tc.tile_pool(name="v2", bufs=NT))
        ptp = ctx.enter_context(tc.tile_pool(name="pT", bufs=2))
        aop = ctx.enter_context(tc.tile_pool(name="aoT", bufs=CT))
        wqkp = ctx.enter_context(tc.tile_pool(name="wqk", bufs=4))
        whp = ctx.enter_context(tc.tile_pool(name="whalf", bufs=2))
        yp = ctx.enter_context(tc.tile_pool(name="ysb", bufs=2))
        rbp = ctx.enter_context(tc.tile_pool(name="rbc", bufs=1))
        aop65 = ctx.enter_context(tc.tile_pool(name="ao65", bufs=4))
        rcolp = ctx.enter_context(tc.tile_pool(name="rcol", bufs=4))
        rdp = ctx.enter_context(tc.tile_pool(name="rdram", bufs=8, space="DRAM"))
        projps = ctx.enter_context(tc.tile_pool(name="projps", bufs=2, space="PSUM"))
        spool = ctx.enter_context(tc.tile_pool(name="spool", bufs=2, space="PSUM"))
        avps = ctx.enter_context(tc.tile_pool(name="avps", bufs=2, space="PSUM"))

        # ---------------- constants ----------------
        ident = const.tile([128, 128], f32)
        make_identity(nc, ident)

        ones_f = const.tile([1, 128], f32)
        nc.vector.memset(ones_f, 1.0)
        ones_col = ones_f.bitcast(f32r)          # 1.0 is exact in f32r
        ones16 = const.tile([128, H], f32)
        nc.vector.memset(ones16, 1.0)
        ones11 = const.tile([1, 1], f32)
        nc.vector.memset(ones11, 1.0)
        ones_bf = const.tile([1, 128], bf16)
        nc.vector.tensor_copy(ones_bf[:], ones_f[:])
        bo_bf = const.tile([1, C], bf16)

        # mask bias columns: [128, NT]  (partition p, col st) = (mask-1)*NEG
        m_t = const.tile([128, NT], i32)
        nc.sync.dma_start(out=m_t, in_=m_d.ap().rearrange("(t p) -> p t", p=128))
        mb = const.tile([128, NT], f32)
        nc.vector.tensor_scalar(mb[:], m_t[:], -1.0, NEG,
                                op0=mybir.AluOpType.add, op1=mybir.AluOpType.mult)

        # per-partition bias columns for q/k: [128, CT] col ct = bias[ct*128+p]
        bq_t = const.tile([128, CT], f32)
        nc.sync.dma_start(out=bq_t, in_=bq_d.ap().rearrange("(t p) -> p t", p=128))
        bk_t = const.tile([128, CT], f32)
        nc.sync.dma_start(out=bk_t, in_=bk_d.ap().rearrange("(t p) -> p t", p=128))
        # bias rows for v/o rank-1 accumulation
        bv_t = const.tile([1, C], f32r)
        nc.sync.dma_start(out=bv_t, in_=bv_d.ap().bitcast(f32r))
        bo_t = const.tile([1, C], f32)
        nc.sync.dma_start(out=bo_t, in_=bo_d.ap())
        nc.vector.tensor_copy(bo_bf[:], bo_t[:])

        # PE warmup: ~4us of dummy matmuls on the identity so the HAM
        # clock-gate reaches 2.4 GHz before the transposes/projections start
        warm_ps = projps.tile([128, 512], f32, tag="proj", name="warm")
        for w in range(24):
            nc.tensor.matmul(warm_ps[:, 0:128], ident[:], ident[:],
                             start=True, stop=True)

        # ---------------- phase 0: load x, transpose to xT ----------------
        xT = []
        for ct in range(CT):
            xT.append(xtp.tile([128, N], f32r, tag="xT", name=f"xT{ct}"))
        xn = []
        for t in range(NT):
            xt_ = xnp.tile([128, C], f32, tag="xn")
            nc.sync.dma_start(out=xt_, in_=x_d.ap()[t * 128:(t + 1) * 128, :])
            xn.append(xt_)
        for j in range(2):           # two groups of 4 seq tiles
            for ct in range(CT):
                trp = projps.tile([128, 512], f32, tag="proj")
                for k in range(4):
                    t = 4 * j + k
                    nc.tensor.transpose(trp[:, k * 128:(k + 1) * 128],
                                        xn[t][:, ct * 128:(ct + 1) * 128],
                                        ident[:])
                nc.vector.tensor_copy(xT[ct][:, j * 512:(j + 1) * 512], trp[:])

        # ---------------- phase a: V projection -> v'' (bf16) ----------------
        v2 = []
        for nt in range(NT):
            v2.append(v2p.tile([128, H, HD], bf16, tag="v2", name=f"v2_{nt}"))
        for qtr in range(4):
            wv_t = whp.tile([128, CT, 256], f32r, tag="whalf")
            nc.scalar.dma_start(
                out=wv_t,
                in_=wv_d.ap().bitcast(f32r).rearrange("(kt p) c -> p kt c", p=128)
                [:, :, qtr * 256:(qtr + 1) * 256])
            for nt in range(NT):
                pv = projps.tile([128, 256], f32, tag="proj")
                nc.tensor.matmul(pv[:], ones_col,
                                 bv_t[:, qtr * 256:(qtr + 1) * 256],
                                 start=True, stop=False)
                for kt in range(CT):
                    nc.tensor.matmul(pv[:], xT[kt][:, nt * 128:(nt + 1) * 128],
                                     wv_t[:, kt, :],
                                     start=False, stop=(kt == CT - 1))
                nc.vector.tensor_copy(
                    v2[nt][:, qtr * 4:(qtr + 1) * 4, 0:D],
                    pv[:].rearrange("p (h d) -> p h d", d=D))
        for nt in range(NT):
            nc.vector.tensor_copy(
                v2[nt][:, :, D:HD],
                ones16.rearrange("p (h one) -> p h one", one=1))

        # prefetch the first half of Wo now (gpsimd cast-DMA f32->bf16) so
        # it is resident long before the output projection needs it
        wo_ts = {}
        for qtr in range(2):
            wo_ts[qtr] = whp.tile([128, CT, 256], bf16, tag="whalf",
                                  name=f"wo{qtr}")
            nc.gpsimd.dma_start(
                out=wo_ts[qtr],
                in_=wo_d.ap().rearrange("(kt p) c -> p kt c", p=128)
                [:, :, qtr * 256:(qtr + 1) * 256])

        # ---------------- phase b: per channel-tile: q/k proj + attention ----
        aoT = []
        for ct in range(CT):
            aoT.append(aop.tile([128, N], bf16, tag="aoT", name=f"aoT{ct}"))

        def recip_normalize(ct, hh, ao65s):
            # denominator row (row 64 of each ao65 half) -> psum COLUMNS via
            # eight K=1 matmuls so the reciprocal runs partition-parallel;
            # then one store + one partition-broadcast DMA (SP queue, kept
            # clear of the bulky weight loads which use the ACT queue).
            dcol = projps.tile([128, 8], f32, tag="proj", name=f"dc{ct}_{hh}")
            for e in range(8):
                nc.tensor.matmul(
                    dcol[:, e:e + 1],
                    ao65s[e // 4][64:65, (e % 4) * 128:(e % 4 + 1) * 128],
                    ones16[64:65, 0:1],
                    start=True, stop=True)
            rcol = rcolp.tile([128, 8], f32, tag="rcol", name=f"rc{ct}_{hh}")
            nc.vector.reciprocal(rcol[:], dcol[:])
            r_dram = rdp.tile([1, N], f32, tag="rdram", name=f"rd{ct}_{hh}")
            nc.sync.dma_start(
                out=r_dram[0, :].rearrange("(e p) -> p e", p=128),
                in_=rcol[:])
            r_bc = rbp.tile([64, N], f32, tag="rbc")
            nc.sync.dma_start(out=r_bc[:],
                              in_=r_dram[0:1, :].partition_broadcast(64))
            for half in range(2):
                nc.vector.tensor_mul(
                    aoT[ct][hh * 64:hh * 64 + 64,
                            half * 512:(half + 1) * 512],
                    ao65s[half][0:64, :],
                    r_bc[:, half * 512:(half + 1) * 512])

        def qk_dma(q):
            # one quarter (256 cols = 2 channel tiles) of Wq/Wk, 1KB bursts
            wq_t = wqkp.tile([128, CT, 256], f32r, tag="wqk", name=f"wq{q}")
            nc.scalar.dma_start(
                out=wq_t,
                in_=wq_d.ap().bitcast(f32r).rearrange("(kt p) c -> p kt c", p=128)
                [:, :, q * 256:(q + 1) * 256])
            wk_t = wqkp.tile([128, CT, 256], f32r, tag="wqk", name=f"wk{q}")
            nc.scalar.dma_start(
                out=wk_t,
                in_=wk_d.ap().bitcast(f32r).rearrange("(kt p) c -> p kt c", p=128)
                [:, :, q * 256:(q + 1) * 256])
            return wq_t, wk_t

        def qk_proj_ops(ct, wq_t, wk_t):
            """Return (qT, kT, ops): ops are deferred closures, executed in
            order, that emit the projection matmuls + copies one at a time so
            they can be interleaved into the scores/exp loop of the previous
            channel tile (keeps the PE busy while ScalarE runs exp)."""
            qT = qkp.tile([128, N], bf16, tag="qkT", name=f"qT{ct}")
            kT = qkp.tile([128, N], bf16, tag="qkT", name=f"kT{ct}")
            ops = []
            state = {}
            for half in range(2):
                for w_t, b_col, dst in ((wq_t, bq_t, qT), (wk_t, bk_t, kT)):
                    def mk_group(half=half, w_t=w_t, b_col=b_col, dst=dst):
                        def alloc():
                            state[(id(w_t), half)] = projps.tile(
                                [128, 512], f32, tag="proj", name="pqk")
                        return alloc
                    alloc = mk_group()
                    c0 = (ct % 2) * 128
                    for kt in range(CT):
                        def mm(kt=kt, half=half, w_t=w_t, alloc=alloc, c0=c0):
                            if kt == 0:
                                alloc()
                            p = state[(id(w_t), half)]
                            nc.tensor.matmul(
                                p[:], w_t[:, kt, c0:c0 + 128],
                                xT[kt][:, half * 512:(half + 1) * 512],
                                start=(kt == 0), stop=(kt == CT - 1))
                        ops.append(mm)
                    def cp(half=half, w_t=w_t, b_col=b_col, dst=dst):
                        p = state[(id(w_t), half)]
                        nc.vector.tensor_scalar_add(
                            dst[:, half * 512:(half + 1) * 512], p[:],
                            b_col[:, ct:ct + 1])
                    ops.append(cp)
            return qT, kT, ops

        wq_quarters = {0: qk_dma(0)}
        qT0, kT0, ops0 = qk_proj_ops(0, *wq_quarters[0])
        for op in ops0:
            op()
        qk_cur = (qT0, kT0)
        next_ops = []
        for ct in range(CT):
            qT, kT = qk_cur
            # prefetch the weight quarter two channel-tiles ahead
            nq = (ct + 2) // 2
            if ct % 2 == 0 and ct + 2 < CT and nq not in wq_quarters:
                wq_quarters[nq] = qk_dma(nq)
            if ct + 1 < CT:
                qTn, kTn, next_ops = qk_proj_ops(ct + 1,
                                                 *wq_quarters[(ct + 1) // 2])
            else:
                qTn = kTn = None
                next_ops = []
            # scores + exp for the 2 heads of this ct, st-wise; AV half-0
            # accumulation chunks trail the exp by one seq tile so the PE
            # never waits on ScalarE.
            pts = []
            for hh in range(2):
                pt = ptp.tile([128, NT, N], bf16, tag="pT", name=f"pT{ct}_{hh}")
                pts.append(pt)
            av0 = []
            for hh in range(2):
                av0.append(avps.tile([65, 512], f32, tag="av",
                                     name=f"av0_{ct}_{hh}"))

            def av0_chunk(st):
                for hh in range(2):
                    nc.tensor.matmul(
                        av0[hh][:],
                        v2[st][:, 2 * ct + hh, :],
                        pts[hh][:, st, 0:512],
                        start=(st == 0), stop=(st == NT - 1))

            for st in range(NT):
                for hh in range(2):
                    r0, r1 = hh * 64, hh * 64 + 64
                    ps = spool.tile([128, N], f32, tag="scores")
                    for half in range(2):
                        nc.tensor.matmul(
                            ps[:, half * 512:(half + 1) * 512],
                            kT[r0:r1, st * 128:(st + 1) * 128],
                            qT[r0:r1, half * 512:(half + 1) * 512],
                            start=True, stop=True)
                    nc.scalar.activation(out=pts[hh][:, st, :], in_=ps[:],
                                         func=mybir.ActivationFunctionType.Exp,
                                         bias=mb[:, st:st + 1], scale=SCALE)
                if st > 1:
                    av0_chunk(st - 2)   # 2 tiles behind: exp surely drained
                # interleave ~5 of the next ct's projection ops to keep the
                # PE fed while ScalarE churns through the exps
                for _ in range(6):
                    if next_ops:
                        next_ops.pop(0)()
            av0_chunk(NT - 2)
            while next_ops:
                next_ops.pop(0)()
            av0_chunk(NT - 1)
            if ct + 1 < CT:
                qk_cur = (qTn, kTn)

            ao65s = {}
            for hh in range(2):
                t = aop65.tile([65, 512], f32, tag="ao65",
                               name=f"ao65_{ct}_{hh}_0")
                nc.vector.tensor_copy(t[:], av0[hh][:])   # frees the bank
                ao65s[hh] = [t]
            # AV half-1: contiguous PE block (exp for this ct already done)
            for hh in range(2):
                av1 = avps.tile([65, 512], f32, tag="av",
                                name=f"av1_{ct}_{hh}")
                for st in range(NT):
                    nc.tensor.matmul(
                        av1[:],
                        v2[st][:, 2 * ct + hh, :],
                        pts[hh][:, st, 512:1024],
                        start=(st == 0), stop=(st == NT - 1))
                t = aop65.tile([65, 512], f32, tag="ao65",
                               name=f"ao65_{ct}_{hh}_1")
                nc.vector.tensor_copy(t[:], av1[:])
                ao65s[hh].append(t)
            for hh in range(2):
                recip_normalize(ct, hh, ao65s[hh])

        # ---------------- phase c: output projection ----------------
        for qtr in range(4):
            if qtr in wo_ts:
                wo_t = wo_ts[qtr]
            else:
                wo_t = whp.tile([128, CT, 256], bf16, tag="whalf",
                                name=f"wo{qtr}")
                nc.gpsimd.dma_start(
                    out=wo_t,
                    in_=wo_d.ap().rearrange(
                        "(kt p) c -> p kt c", p=128)
                    [:, :, qtr * 256:(qtr + 1) * 256])
            for nt in range(NT):
                py = projps.tile([128, 256], f32, tag="proj")
                nc.tensor.matmul(py[:], ones_bf[:],
                                 bo_bf[:, qtr * 256:(qtr + 1) * 256],
                                 start=True, stop=False)
                for kt in range(CT):
                    nc.tensor.matmul(py[:], aoT[kt][:, nt * 128:(nt + 1) * 128],
                                     wo_t[:, kt, :],
                                     start=False, stop=(kt == CT - 1))
                y = yp.tile([128, 256], f32, tag="ysb")
                nc.vector.tensor_copy(y[:], py[:])
                nc.sync.dma_start(
                    out=out_d.ap()[nt * 128:(nt + 1) * 128,
                                   qtr * 256:(qtr + 1) * 256],
                    in_=y[:])

    nc.compile()
    return nc


_NC = None


def _get_nc():
    global _NC
    if _NC is None:
        _NC = _build()
    return _NC


def _in_maps(inputs):
    q = np.ascontiguousarray(np.asarray(inputs["query"], dtype=np.float32))
    mask = np.ascontiguousarray(np.asarray(inputs["mask"], dtype=np.int32))
    shared = {
        "Wq": np.ascontiguousarray(np.asarray(inputs["Wq"], dtype=np.float32)),
        "Wk": np.ascontiguousarray(np.asarray(inputs["Wk"], dtype=np.float32)),
        "Wv": np.ascontiguousarray(np.asarray(inputs["Wv"], dtype=np.float32)),
        "Wo": np.ascontiguousarray(np.asarray(inputs["Wo"], dtype=np.float32)),
        "bq": np.ascontiguousarray(np.asarray(inputs["bq"], dtype=np.float32)),
        "bk": np.ascontiguousarray(np.asarray(inputs["bk"], dtype=np.float32)),
        "bv": np.ascontiguousarray(np.asarray(inputs["bv"], dtype=np.float32)),
        "bo": np.ascontiguousarray(np.asarray(inputs["bo"], dtype=np.float32)),
    }
    in_maps = []
    for b in range(B):
        m = {"x": np.ascontiguousarray(q[b]), "mask": np.ascontiguousarray(mask[b])}
        m.update(shared)
        in_maps.append(m)
    return in_maps


def kernel(**inputs):
    nc = _get_nc()
    res = bass_utils.run_bass_kernel_spmd(nc, _in_maps(inputs),
                                          core_ids=list(range(B)))
    return np.stack([r["out"] for r in res.results]).astype(np.float32)


if __name__ == "__main__":
    rng = np.random.default_rng(0)
    inputs = {
        "query": rng.standard_normal((B, N, C), dtype=np.float32),
        "mask": rng.integers(0, 2, (B, N)).astype(np.int32),
        "Wq": (rng.standard_normal((C, C), dtype=np.float32) * C ** -0.5),
        "bq": np.zeros(C, np.float32),
        "Wk": (rng.standard_normal((C, C), dtype=np.float32) * C ** -0.5),
        "bk": np.zeros(C, np.float32),
        "Wv": (rng.standard_normal((C, C), dtype=np.float32) * C ** -0.5),
        "bv": np.zeros(C, np.float32),
        "Wo": (rng.standard_normal((C, C), dtype=np.float32) * C ** -0.5),
        "bo": np.zeros(C, np.float32),
    }
    out = kernel(**inputs)
    # numpy reference
    def ref(q, mask, Wq, bq, Wk, bk, Wv, bv, Wo, bo):
        Bq, Nq, Cq = q.shape
        qq = (q @ Wq + bq).reshape(Bq, Nq, H, D).transpose(0, 2, 1, 3)
        kk = (q @ Wk + bk).reshape(Bq, Nq, H, D).transpose(0, 2, 1, 3)
        vv = (q @ Wv + bv).reshape(Bq, Nq, H, D).transpose(0, 2, 1, 3)
        at = np.einsum("bhnd,bhsd->bhns", qq, kk) * SCALE
        at = np.where(mask[:, None, None, :] == 0, -np.inf, at)
        at = at - at.max(-1, keepdims=True)
        e = np.exp(at)
        p = e / e.sum(-1, keepdims=True)
        o = np.einsum("bhns,bhsd->bhnd", p, vv)
        o = o.transpose(0, 2, 1, 3).reshape(Bq, Nq, Cq)
        return o @ Wo + bo
    expected = ref(**{k: inputs[k] for k in
                      ["query", "mask", "Wq", "bq", "Wk", "bk", "Wv", "bv", "Wo", "bo"]}
                   .values()) if False else None
    expected = ref(inputs["query"], inputs["mask"], inputs["Wq"], inputs["bq"],
                   inputs["Wk"], inputs["bk"], inputs["Wv"], inputs["bv"],
                   inputs["Wo"], inputs["bo"])
    err = np.abs(out - expected).max() / np.abs(expected).max()
    print("self-test rel err:", err)


# revision 26
# speedup vs baseline: 1.0505x; 1.0005x over previous
"""Multi-head attention (B=8, N=1024, C=1024, H=16, D=64) on 8 TRN2 NeuronCores.

Strategy: pure data parallelism — one batch element per core, weights
replicated, no collectives.  Per-core dataflow (all layouts chosen so every
matmul contracts over the partition axis):

  x [N,C]  --PE transpose-->  xT [C,N]
  qT = Wq^T@. : lhsT=Wq tile,  rhs=xT      -> [C,N]   (float32r matmuls)
  kT =                        same          -> [C,N]
  v  = x@Wv  : lhsT=xT tile,   rhs=Wv      -> [N,C]   -> v'' bf16 [N, 16*(D+1)]
               (per head: 64 v columns + a ones column for the softmax denom)
  per head h (paired 2 per channel-tile, PE row-tiling K=64):
    scores^T[s,n] = kT_h^T @ qT_h          (K=64)
    p^T = exp(scores^T * scale + mask_bias[s])   (ScalarE, bias kills masked
                                                  KEY rows exactly -> 0)
    o^T[0:64] , denom[64] = v''_h^T @ p^T  (K=1024, m=65, bf16)
    ao^T_h = o^T * (1/denom broadcast)     (DMA partition-broadcast of recip)
  y = ao@Wo + bo : lhsT=aoT tile, rhs=Wo   -> [N,C]

Biases are applied: bq/bk as per-partition adds on the qT/kT copies, bv/bo as
rank-1 (ones x bias) matmul accumulations into PSUM.
"""
import numpy as np

import concourse.bass as bass
import concourse.mybir as mybir
import concourse.tile as tile
from concourse import bacc
from concourse import bass_utils
from concourse.masks import make_identity

f32 = mybir.dt.float32
f32r = mybir.dt.float32r
bf16 = mybir.dt.bfloat16
i32 = mybir.dt.int32

B, N, C, H, D = 8, 1024, 1024, 16, 64
NT = N // 128          # seq tiles
CT = C // 128          # channel tiles
HD = D + 1             # head slice width in v'' (64 v cols + ones col)
SCALE = float(D) ** -0.5
NEG = 30000.0          # exp(-30000) == 0.0 exactly in fp32


def _build():
    nc = bacc.Bacc("TRN2", target_bir_lowering=False, debug=False)

    x_d = nc.declare_dram_parameter("x", [N, C], f32, isOutput=False)
    m_d = nc.declare_dram_parameter("mask", [N], i32, isOutput=False)
    wq_d = nc.declare_dram_parameter("Wq", [C, C], f32, isOutput=False)
    wk_d = nc.declare_dram_parameter("Wk", [C, C], f32, isOutput=False)
    wv_d = nc.declare_dram_parameter("Wv", [C, C], f32, isOutput=False)
    wo_d = nc.declare_dram_parameter("Wo", [C, C], f32, isOutput=False)
    bq_d = nc.declare_dram_parameter("bq", [C], f32, isOutput=False)
    bk_d = nc.declare_dram_parameter("bk", [C], f32, isOutput=False)
    bv_d = nc.declare_dram_parameter("bv", [C], f32, isOutput=False)
    bo_d = nc.declare_dram_parameter("bo", [C], f32, isOutput=False)
    out_d = nc.declare_dram_parameter("out", [N, C], f32, isOutput=True)

    from contextlib import ExitStack
    with ExitStack() as ctx:
        tc = ctx.enter_context(tile.TileContext(nc))
        const = ctx.enter_context(tc.tile_pool(name="const", bufs=1))
        xnp = ctx.enter_context(tc.tile_pool(name="xn", bufs=4))
        xtp = ctx.enter_context(tc.tile_pool(name="xT", bufs=CT))
        qkp = ctx.enter_context(tc.tile_pool(name="qkT", bufs=4))
        v2p = ctx.enter_context(tc.tile_pool(name="v2", bufs=NT))
        ptp = ctx.enter_context(tc.tile_pool(name="pT", bufs=2))
        aop = ctx.enter_context(tc.tile_pool(name="aoT", bufs=CT))
        wqkp = ctx.enter_context(tc.tile_pool(name="wqk", bufs=4))
        whp = ctx.enter_context(tc.tile_pool(name="whalf", bufs=2))
        yp = ctx.enter_context(tc.tile_pool(name="ysb", bufs=2))
        rbp = ctx.enter_context(tc.tile_pool(name="rbc", bufs=2))
        aop65 = ctx.enter_context(tc.tile_pool(name="ao65", bufs=4))
        rcolp = ctx.enter_context(tc.tile_pool(name="rcol", bufs=4))
        rdp = ctx.enter_context(tc.tile_pool(name="rdram", bufs=8, space="DRAM"))
        projps = ctx.enter_context(tc.tile_pool(name="projps", bufs=2, space="PSUM"))
        spool = ctx.enter_context(tc.tile_pool(name="spool", bufs=2, space="PSUM"))
        avps = ctx.enter_context(tc.tile_pool(name="avps", bufs=2, space="PSUM"))

        # ---------------- constants ----------------
        ident = const.tile([128, 128], f32)
        make_identity(nc, ident)

        ones_f = const.tile([1, 128], f32)
        nc.vector.memset(ones_f, 1.0)
        ones_col = ones_f.bitcast(f32r)          # 1.0 is exact in f32r
        ones16 = const.tile([128, H], f32)
        nc.vector.memset(ones16, 1.0)
        ones11 = const.tile([1, 1], f32)
        nc.vector.memset(ones11, 1.0)
        ones_bf = const.tile([1, 128], bf16)
        nc.vector.tensor_copy(ones_bf[:], ones_f[:])
        bo_bf = const.tile([1, C], bf16)

        # mask bias columns: [128, NT]  (partition p, col st) = (mask-1)*NEG
        m_t = const.tile([128, NT], i32)
        nc.sync.dma_start(out=m_t, in_=m_d.ap().rearrange("(t p) -> p t", p=128))
        mb = const.tile([128, NT], f32)
        nc.vector.tensor_scalar(mb[:], m_t[:], -1.0, NEG,
                                op0=mybir.AluOpType.add, op1=mybir.AluOpType.mult)

        # per-partition bias columns for q/k: [128, CT] col ct = bias[ct*128+p]
        bq_t = const.tile([128, CT], f32)
        nc.sync.dma_start(out=bq_t, in_=bq_d.ap().rearrange("(t p) -> p t", p=128))
        bk_t = const.tile([128, CT], f32)
        nc.sync.dma_start(out=bk_t, in_=bk_d.ap().rearrange("(t p) -> p t", p=128))
        # bias rows for v/o rank-1 accumulation
        bv_t = const.tile([1, C], f32r)
        nc.sync.dma_start(out=bv_t, in_=bv_d.ap().bitcast(f32r))
        bo_t = const.tile([1, C], f32)
        nc.sync.dma_start(out=bo_t, in_=bo_d.ap())
        nc.vector.tensor_copy(bo_bf[:], bo_t[:])

        # PE warmup: ~4us of dummy matmuls on the identity so the HAM
        # clock-gate reaches 2.4 GHz before the transposes/projections start
        warm_ps = projps.tile([128, 512], f32, tag="proj", name="warm")
        for w in range(24):
            nc.tensor.matmul(warm_ps[:, 0:128], ident[:], ident[:],
                             start=True, stop=True)

        # ---------------- phase 0: load x, transpose to xT ----------------
        xT = []
        for ct in range(CT):
            xT.append(xtp.tile([128, N], f32r, tag="xT", name=f"xT{ct}"))
        xn = []
        for t in range(NT):
            xt_ = xnp.tile([128, C], f32, tag="xn")
            nc.sync.dma_start(out=xt_, in_=x_d.ap()[t * 128:(t + 1) * 128, :])
            xn.append(xt_)
        for j in range(2):           # two groups of 4 seq tiles
            for ct in range(CT):
                trp = projps.tile([128, 512], f32, tag="proj")
                for k in range(4):
                    t = 4 * j + k
                    nc.tensor.transpose(trp[:, k * 128:(k + 1) * 128],
                                        xn[t][:, ct * 128:(ct + 1) * 128],
                                        ident[:])
                nc.vector.tensor_copy(xT[ct][:, j * 512:(j + 1) * 512], trp[:])

        # ---------------- phase a: V projection -> v'' (bf16) ----------------
        v2 = []
        for nt in range(NT):
            v2.append(v2p.tile([128, H, HD], bf16, tag="v2", name=f"v2_{nt}"))
        for qtr in range(4):
            wv_t = whp.tile([128, CT, 256], f32r, tag="whalf")
            nc.scalar.dma_start(
                out=wv_t,
                in_=wv_d.ap().bitcast(f32r).rearrange("(kt p) c -> p kt c", p=128)
                [:, :, qtr * 256:(qtr + 1) * 256])
            for nt in range(NT):
                pv = projps.tile([128, 256], f32, tag="proj")
                nc.tensor.matmul(pv[:], ones_col,
                                 bv_t[:, qtr * 256:(qtr + 1) * 256],
                                 start=True, stop=False)
                for kt in range(CT):
                    nc.tensor.matmul(pv[:], xT[kt][:, nt * 128:(nt + 1) * 128],
                                     wv_t[:, kt, :],
                                     start=False, stop=(kt == CT - 1))
                nc.vector.tensor_copy(
                    v2[nt][:, qtr * 4:(qtr + 1) * 4, 0:D],
                    pv[:].rearrange("p (h d) -> p h d", d=D))
        for nt in range(NT):
            nc.vector.tensor_copy(
                v2[nt][:, :, D:HD],
                ones16.rearrange("p (h one) -> p h one", one=1))

        # prefetch the first half of Wo now (gpsimd cast-DMA f32->bf16) so
        # it is resident long before the output projection needs it
        wo_ts = {}
        for qtr in range(2):
            wo_ts[qtr] = whp.tile([128, CT, 256], bf16, tag="whalf",
                                  name=f"wo{qtr}")
            nc.gpsimd.dma_start(
                out=wo_ts[qtr],
                in_=wo_d.ap().rearrange("(kt p) c -> p kt c", p=128)
                [:, :, qtr * 256:(qtr + 1) * 256])

        # ---------------- phase b: per channel-tile: q/k proj + attention ----
        aoT = []
        for ct in range(CT):
            aoT.append(aop.tile([128, N], bf16, tag="aoT", name=f"aoT{ct}"))

        def recip_normalize(ct, hh, ao65s):
            # denominator row (row 64 of each ao65 half) -> psum COLUMNS via
            # eight K=1 matmuls so the reciprocal runs partition-parallel;
            # then one store + one partition-broadcast DMA (SP queue, kept
            # clear of the bulky weight loads which use the ACT queue).
            dcol = projps.tile([128, 8], f32, tag="proj", name=f"dc{ct}_{hh}")
            for e in range(8):
                nc.tensor.matmul(
                    dcol[:, e:e + 1],
                    ao65s[e // 4][64:65, (e % 4) * 128:(e % 4 + 1) * 128],
                    ones16[64:65, 0:1],
                    start=True, stop=True)
            rcol = rcolp.tile([128, 8], f32, tag="rcol", name=f"rc{ct}_{hh}")
            nc.vector.reciprocal(rcol[:], dcol[:])
            r_dram = rdp.tile([1, N], f32, tag="rdram", name=f"rd{ct}_{hh}")
            nc.sync.dma_start(
                out=r_dram[0, :].rearrange("(e p) -> p e", p=128),
                in_=rcol[:])
            r_bc = rbp.tile([64, N], f32, tag="rbc")
            nc.sync.dma_start(out=r_bc[:],
                              in_=r_dram[0:1, :].partition_broadcast(64))
            for half in range(2):
                nc.vector.tensor_mul(
                    aoT[ct][hh * 64:hh * 64 + 64,
                            half * 512:(half + 1) * 512],
                    ao65s[half][0:64, :],
                    r_bc[:, half * 512:(half + 1) * 512])

        def qk_dma(q):
            # one quarter (256 cols = 2 channel tiles) of Wq/Wk, 1KB bursts
            wq_t = wqkp.tile([128, CT, 256], f32r, tag="wqk", name=f"wq{q}")
            nc.scalar.dma_start(
                out=wq_t,
                in_=wq_d.ap().bitcast(f32r).rearrange("(kt p) c -> p kt c", p=128)
                [:, :, q * 256:(q + 1) * 256])
            wk_t = wqkp.tile([128, CT, 256], f32r, tag="wqk", name=f"wk{q}")
            nc.scalar.dma_start(
                out=wk_t,
                in_=wk_d.ap().bitcast(f32r).rearrange("(kt p) c -> p kt c", p=128)
                [:, :, q * 256:(q + 1) * 256])
            return wq_t, wk_t

        def qk_proj_ops(ct, wq_t, wk_t):
            """Return (qT, kT, ops): ops are deferred closures, executed in
            order, that emit the projection matmuls + copies one at a time so
            they can be interleaved into the scores/exp loop of the previous
            channel tile (keeps the PE busy while ScalarE runs exp)."""
            qT = qkp.tile([128, N], bf16, tag="qkT", name=f"qT{ct}")
            kT = qkp.tile([128, N], bf16, tag="qkT", name=f"kT{ct}")
            ops = []
            state = {}
            for half in range(2):
                for w_t, b_col, dst in ((wq_t, bq_t, qT), (wk_t, bk_t, kT)):
                    def mk_group(half=half, w_t=w_t, b_col=b_col, dst=dst):
                        def alloc():
                            state[(id(w_t), half)] = projps.tile(
                                [128, 512], f32, tag="proj", name="pqk")
                        return alloc
                    alloc = mk_group()
                    c0 = (ct % 2) * 128
                    for kt in range(CT):
                        def mm(kt=kt, half=half, w_t=w_t, alloc=alloc, c0=c0):
                            if kt == 0:
                                alloc()
                            p = state[(id(w_t), half)]
                            nc.tensor.matmul(
                                p[:], w_t[:, kt, c0:c0 + 128],
                                xT[kt][:, half * 512:(half + 1) * 512],
                                start=(kt == 0), stop=(kt == CT - 1))
                        ops.append(mm)
                    def cp(half=half, w_t=w_t, b_col=b_col, dst=dst):
                        p = state[(id(w_t), half)]
                        nc.vector.tensor_scalar_add(
                            dst[:, half * 512:(half + 1) * 512], p[:],
                            b_col[:, ct:ct + 1])
                    ops.append(cp)
            return qT, kT, ops

        wq_quarters = {0: qk_dma(0)}
        qT0, kT0, ops0 = qk_proj_ops(0, *wq_quarters[0])
        for op in ops0:
            op()
        qk_cur = (qT0, kT0)
        next_ops = []
        for ct in range(CT):
            qT, kT = qk_cur
            # prefetch the weight quarter two channel-tiles ahead
            nq = (ct + 2) // 2
            if ct % 2 == 0 and ct + 2 < CT and nq not in wq_quarters:
                wq_quarters[nq] = qk_dma(nq)
            if ct + 1 < CT:
                qTn, kTn, next_ops = qk_proj_ops(ct + 1,
                                                 *wq_quarters[(ct + 1) // 2])
            else:
                qTn = kTn = None
                next_ops = []
            # scores + exp for the 2 heads of this ct, st-wise; AV half-0
            # accumulation chunks trail the exp by one seq tile so the PE
            # never waits on ScalarE.
            pts = []
            for hh in range(2):
                pt = ptp.tile([128, NT, N], bf16, tag="pT", name=f"pT{ct}_{hh}")
                pts.append(pt)
            av0 = []
            for hh in range(2):
                av0.append(avps.tile([65, 512], f32, tag="av",
                                     name=f"av0_{ct}_{hh}"))

            def av0_chunk(st):
                for hh in range(2):
                    nc.tensor.matmul(
                        av0[hh][:],
                        v2[st][:, 2 * ct + hh, :],
                        pts[hh][:, st, 0:512],
                        start=(st == 0), stop=(st == NT - 1))

            for st in range(NT):
                for hh in range(2):
                    r0, r1 = hh * 64, hh * 64 + 64
                    ps = spool.tile([128, N], f32, tag="scores")
                    for half in range(2):
                        nc.tensor.matmul(
                            ps[:, half * 512:(half + 1) * 512],
                            kT[r0:r1, st * 128:(st + 1) * 128],
                            qT[r0:r1, half * 512:(half + 1) * 512],
                            start=True, stop=True)
                    nc.scalar.activation(out=pts[hh][:, st, :], in_=ps[:],
                                         func=mybir.ActivationFunctionType.Exp,
                                         bias=mb[:, st:st + 1], scale=SCALE)
                if st > 1:
                    av0_chunk(st - 2)   # 2 tiles behind: exp surely drained
                # interleave ~5 of the next ct's projection ops to keep the
                # PE fed while ScalarE churns through the exps
                for _ in range(6):
                    if next_ops:
                        next_ops.pop(0)()
            av0_chunk(NT - 2)
            while next_ops:
                next_ops.pop(0)()
            av0_chunk(NT - 1)
            if ct + 1 < CT:
                qk_cur = (qTn, kTn)

            ao65s = {}
            for hh in range(2):
                t = aop65.tile([65, 512], f32, tag="ao65",
                               name=f"ao65_{ct}_{hh}_0")
                nc.vector.tensor_copy(t[:], av0[hh][:])   # frees the bank
                ao65s[hh] = [t]
            # AV half-1: contiguous PE block (exp for this ct already done)
            for hh in range(2):
                av1 = avps.tile([65, 512], f32, tag="av",
                                name=f"av1_{ct}_{hh}")
                for st in range(NT):
                    nc.tensor.matmul(
                        av1[:],
                        v2[st][:, 2 * ct + hh, :],
                        pts[hh][:, st, 512:1024],
                        start=(st == 0), stop=(st == NT - 1))
                t = aop65.tile([65, 512], f32, tag="ao65",
                               name=f"ao65_{ct}_{hh}_1")
                nc.vector.tensor_copy(t[:], av1[:])
                ao65s[hh].append(t)
            for hh in range(2):
                recip_normalize(ct, hh, ao65s[hh])

        # ---------------- phase c: output projection ----------------
        for qtr in range(4):
            if qtr in wo_ts:
                wo_t = wo_ts[qtr]
            else:
                wo_t = whp.tile([128, CT, 256], bf16, tag="whalf",
                                name=f"wo{qtr}")
                nc.gpsimd.dma_start(
                    out=wo_t,
                    in_=wo_d.ap().rearrange(
                        "(kt p) c -> p kt c", p=128)
                    [:, :, qtr * 256:(qtr + 1) * 256])
            for nt in range(NT):
                py = projps.tile([128, 256], f32, tag="proj")
                nc.tensor.matmul(py[:], ones_bf[:],
                                 bo_bf[:, qtr * 256:(qtr + 1) * 256],
                                 start=True, stop=False)
                for kt in range(CT):
                    nc.tensor.matmul(py[:], aoT[kt][:, nt * 128:(nt + 1) * 128],
                                     wo_t[:, kt, :],
                                     start=False, stop=(kt == CT - 1))
                y = yp.tile([128, 256], f32, tag="ysb")
                nc.vector.tensor_copy(y[:], py[:])
                nc.sync.dma_start(
                    out=out_d.ap()[nt * 128:(nt + 1) * 128,
                                   qtr * 256:(qtr + 1) * 256],
                    in_=y[:])

    nc.compile()
    return nc


_NC = None


def _get_nc():
    global _NC
    if _NC is None:
        _NC = _build()
    return _NC


def _in_maps(inputs):
    q = np.ascontiguousarray(np.asarray(inputs["query"], dtype=np.float32))
    mask = np.ascontiguousarray(np.asarray(inputs["mask"], dtype=np.int32))
    shared = {
        "Wq": np.ascontiguousarray(np.asarray(inputs["Wq"], dtype=np.float32)),
        "Wk": np.ascontiguousarray(np.asarray(inputs["Wk"], dtype=np.float32)),
        "Wv": np.ascontiguousarray(np.asarray(inputs["Wv"], dtype=np.float32)),
        "Wo": np.ascontiguousarray(np.asarray(inputs["Wo"], dtype=np.float32)),
        "bq": np.ascontiguousarray(np.asarray(inputs["bq"], dtype=np.float32)),
        "bk": np.ascontiguousarray(np.asarray(inputs["bk"], dtype=np.float32)),
        "bv": np.ascontiguousarray(np.asarray(inputs["bv"], dtype=np.float32)),
        "bo": np.ascontiguousarray(np.asarray(inputs["bo"], dtype=np.float32)),
    }
    in_maps = []
    for b in range(B):
        m = {"x": np.ascontiguousarray(q[b]), "mask": np.ascontiguousarray(mask[b])}
        m.update(shared)
        in_maps.append(m)
    return in_maps


def kernel(**inputs):
    nc = _get_nc()
    res = bass_utils.run_bass_kernel_spmd(nc, _in_maps(inputs),
                                          core_ids=list(range(B)))
    return np.stack([r["out"] for r in res.results]).astype(np.float32)


if __name__ == "__main__":
    rng = np.random.default_rng(0)
    inputs = {
        "query": rng.standard_normal((B, N, C), dtype=np.float32),
        "mask": rng.integers(0, 2, (B, N)).astype(np.int32),
        "Wq": (rng.standard_normal((C, C), dtype=np.float32) * C ** -0.5),
        "bq": np.zeros(C, np.float32),
        "Wk": (rng.standard_normal((C, C), dtype=np.float32) * C ** -0.5),
        "bk": np.zeros(C, np.float32),
        "Wv": (rng.standard_normal((C, C), dtype=np.float32) * C ** -0.5),
        "bv": np.zeros(C, np.float32),
        "Wo": (rng.standard_normal((C, C), dtype=np.float32) * C ** -0.5),
        "bo": np.zeros(C, np.float32),
    }
    out = kernel(**inputs)
    # numpy reference
    def ref(q, mask, Wq, bq, Wk, bk, Wv, bv, Wo, bo):
        Bq, Nq, Cq = q.shape
        qq = (q @ Wq + bq).reshape(Bq, Nq, H, D).transpose(0, 2, 1, 3)
        kk = (q @ Wk + bk).reshape(Bq, Nq, H, D).transpose(0, 2, 1, 3)
        vv = (q @ Wv + bv).reshape(Bq, Nq, H, D).transpose(0, 2, 1, 3)
        at = np.einsum("bhnd,bhsd->bhns", qq, kk) * SCALE
        at = np.where(mask[:, None, None, :] == 0, -np.inf, at)
        at = at - at.max(-1, keepdims=True)
        e = np.exp(at)
        p = e / e.sum(-1, keepdims=True)
        o = np.einsum("bhns,bhsd->bhnd", p, vv)
        o = o.transpose(0, 2, 1, 3).reshape(Bq, Nq, Cq)
        return o @ Wo + bo
    expected = ref(**{k: inputs[k] for k in
                      ["query", "mask", "Wq", "bq", "Wk", "bk", "Wv", "bv", "Wo", "bo"]}
                   .values()) if False else None
    expected = ref(inputs["query"], inputs["mask"], inputs["Wq"], inputs["bq"],
                   inputs["Wk"], inputs["bk"], inputs["Wv"], inputs["bv"],
                   inputs["Wo"], inputs["bo"])
    err = np.abs(out - expected).max() / np.abs(expected).max()
    print("self-test rel err:", err)


# revision 27
# speedup vs baseline: 1.0700x; 1.0185x over previous
"""Multi-head attention (B=8, N=1024, C=1024, H=16, D=64) on 8 TRN2 NeuronCores.

Strategy: pure data parallelism — one batch element per core, weights
replicated, no collectives.  Per-core dataflow (all layouts chosen so every
matmul contracts over the partition axis):

  x [N,C]  --PE transpose-->  xT [C,N]
  qT = Wq^T@. : lhsT=Wq tile,  rhs=xT      -> [C,N]   (float32r matmuls)
  kT =                        same          -> [C,N]
  v  = x@Wv  : lhsT=xT tile,   rhs=Wv      -> [N,C]   -> v'' bf16 [N, 16*(D+1)]
               (per head: 64 v columns + a ones column for the softmax denom)
  per head h (paired 2 per channel-tile, PE row-tiling K=64):
    scores^T[s,n] = kT_h^T @ qT_h          (K=64)
    p^T = exp(scores^T * scale + mask_bias[s])   (ScalarE, bias kills masked
                                                  KEY rows exactly -> 0)
    o^T[0:64] , denom[64] = v''_h^T @ p^T  (K=1024, m=65, bf16)
    ao^T_h = o^T * (1/denom broadcast)     (DMA partition-broadcast of recip)
  y = ao@Wo + bo : lhsT=aoT tile, rhs=Wo   -> [N,C]

Biases are applied: bq/bk as per-partition adds on the qT/kT copies, bv/bo as
rank-1 (ones x bias) matmul accumulations into PSUM.
"""
import numpy as np

import concourse.bass as bass
import concourse.mybir as mybir
import concourse.tile as tile
from concourse import bacc
from concourse import bass_utils
from concourse.masks import make_identity

f32 = mybir.dt.float32
f32r = mybir.dt.float32r
bf16 = mybir.dt.bfloat16
i32 = mybir.dt.int32

B, N, C, H, D = 8, 1024, 1024, 16, 64
NT = N // 128          # seq tiles
CT = C // 128          # channel tiles
HD = D + 1             # head slice width in v'' (64 v cols + ones col)
SCALE = float(D) ** -0.5
NEG = 30000.0          # exp(-30000) == 0.0 exactly in fp32


def _build():
    nc = bacc.Bacc("TRN2", target_bir_lowering=False, debug=False)

    x_d = nc.declare_dram_parameter("x", [N, C], f32, isOutput=False)
    m_d = nc.declare_dram_parameter("mask", [N], i32, isOutput=False)
    wq_d = nc.declare_dram_parameter("Wq", [C, C], f32, isOutput=False)
    wk_d = nc.declare_dram_parameter("Wk", [C, C], f32, isOutput=False)
    wv_d = nc.declare_dram_parameter("Wv", [C, C], f32, isOutput=False)
    wo_d = nc.declare_dram_parameter("Wo", [C, C], f32, isOutput=False)
    bq_d = nc.declare_dram_parameter("bq", [C], f32, isOutput=False)
    bk_d = nc.declare_dram_parameter("bk", [C], f32, isOutput=False)
    bv_d = nc.declare_dram_parameter("bv", [C], f32, isOutput=False)
    bo_d = nc.declare_dram_parameter("bo", [C], f32, isOutput=False)
    out_d = nc.declare_dram_parameter("out", [N, C], f32, isOutput=True)

    from contextlib import ExitStack
    with ExitStack() as ctx:
        tc = ctx.enter_context(tile.TileContext(nc))
        const = ctx.enter_context(tc.tile_pool(name="const", bufs=1))
        xnp = ctx.enter_context(tc.tile_pool(name="xn", bufs=4))
        xtp = ctx.enter_context(tc.tile_pool(name="xT", bufs=CT))
        qkp = ctx.enter_context(tc.tile_pool(name="qkT", bufs=4))
        v2p = ctx.enter_context(tc.tile_pool(name="v2", bufs=NT))
        ptp = ctx.enter_context(tc.tile_pool(name="pT", bufs=2))
        aop = ctx.enter_context(tc.tile_pool(name="aoT", bufs=CT))
        wqkp = ctx.enter_context(tc.tile_pool(name="wqk", bufs=4))
        whp = ctx.enter_context(tc.tile_pool(name="whalf", bufs=2))
        yp = ctx.enter_context(tc.tile_pool(name="ysb", bufs=2))
        rbp = ctx.enter_context(tc.tile_pool(name="rbc", bufs=2))
        aop65 = ctx.enter_context(tc.tile_pool(name="ao65", bufs=4))
        rcolp = ctx.enter_context(tc.tile_pool(name="rcol", bufs=4))
        rdp = ctx.enter_context(tc.tile_pool(name="rdram", bufs=8, space="DRAM"))
        projps = ctx.enter_context(tc.tile_pool(name="projps", bufs=2, space="PSUM"))
        spool = ctx.enter_context(tc.tile_pool(name="spool", bufs=2, space="PSUM"))
        avps = ctx.enter_context(tc.tile_pool(name="avps", bufs=2, space="PSUM"))

        # x tiles first: their DMAs must lead the SP queue so the
        # transposes can start immediately (the strided bias/mask gathers
        # below are slow and not needed until much later)
        xn = []
        for t in range(NT):
            xt_ = xnp.tile([128, C], f32, tag="xn")
            nc.sync.dma_start(out=xt_, in_=x_d.ap()[t * 128:(t + 1) * 128, :])
            xn.append(xt_)

        # ---------------- constants ----------------
        ident = const.tile([128, 128], f32)
        make_identity(nc, ident)

        ones_f = const.tile([1, 128], f32)
        nc.vector.memset(ones_f, 1.0)
        ones_col = ones_f.bitcast(f32r)          # 1.0 is exact in f32r
        ones16 = const.tile([128, H], f32)
        nc.vector.memset(ones16, 1.0)
        ones11 = const.tile([1, 1], f32)
        nc.vector.memset(ones11, 1.0)
        ones_bf = const.tile([1, 128], bf16)
        nc.vector.tensor_copy(ones_bf[:], ones_f[:])
        bo_bf = const.tile([1, C], bf16)

        # mask bias columns: [128, NT]  (partition p, col st) = (mask-1)*NEG
        m_t = const.tile([128, NT], i32)
        nc.sync.dma_start(out=m_t, in_=m_d.ap().rearrange("(t p) -> p t", p=128))
        mb = const.tile([128, NT], f32)
        nc.vector.tensor_scalar(mb[:], m_t[:], -1.0, NEG,
                                op0=mybir.AluOpType.add, op1=mybir.AluOpType.mult)

        # per-partition bias columns for q/k: [128, CT] col ct = bias[ct*128+p]
        bq_t = const.tile([128, CT], f32)
        nc.sync.dma_start(out=bq_t, in_=bq_d.ap().rearrange("(t p) -> p t", p=128))
        bk_t = const.tile([128, CT], f32)
        nc.sync.dma_start(out=bk_t, in_=bk_d.ap().rearrange("(t p) -> p t", p=128))
        # bias rows for v/o rank-1 accumulation
        bv_t = const.tile([1, C], f32r)
        nc.sync.dma_start(out=bv_t, in_=bv_d.ap().bitcast(f32r))
        bo_t = const.tile([1, C], f32)
        nc.sync.dma_start(out=bo_t, in_=bo_d.ap())
        nc.vector.tensor_copy(bo_bf[:], bo_t[:])

        # PE warmup: ~4us of dummy matmuls on the identity so the HAM
        # clock-gate reaches 2.4 GHz before the transposes/projections start
        warm_ps = projps.tile([128, 512], f32, tag="proj", name="warm")
        for w in range(24):
            nc.tensor.matmul(warm_ps[:, 0:128], ident[:], ident[:],
                             start=True, stop=True)

        # ---------------- phase 0: load x, transpose to xT ----------------
        xT = []
        for ct in range(CT):
            xT.append(xtp.tile([128, N], f32r, tag="xT", name=f"xT{ct}"))
        for j in range(2):           # two groups of 4 seq tiles
            for ct in range(CT):
                trp = projps.tile([128, 512], f32, tag="proj")
                for k in range(4):
                    t = 4 * j + k
                    nc.tensor.transpose(trp[:, k * 128:(k + 1) * 128],
                                        xn[t][:, ct * 128:(ct + 1) * 128],
                                        ident[:])
                nc.vector.tensor_copy(xT[ct][:, j * 512:(j + 1) * 512], trp[:])

        # ---------------- phase a: V projection -> v'' (bf16) ----------------
        v2 = []
        for nt in range(NT):
            v2.append(v2p.tile([128, H, HD], bf16, tag="v2", name=f"v2_{nt}"))
        for qtr in range(4):
            wv_t = whp.tile([128, CT, 256], f32r, tag="whalf")
            nc.scalar.dma_start(
                out=wv_t,
                in_=wv_d.ap().bitcast(f32r).rearrange("(kt p) c -> p kt c", p=128)
                [:, :, qtr * 256:(qtr + 1) * 256])
            for nt in range(NT):
                pv = projps.tile([128, 256], f32, tag="proj")
                nc.tensor.matmul(pv[:], ones_col,
                                 bv_t[:, qtr * 256:(qtr + 1) * 256],
                                 start=True, stop=False)
                for kt in range(CT):
                    nc.tensor.matmul(pv[:], xT[kt][:, nt * 128:(nt + 1) * 128],
                                     wv_t[:, kt, :],
                                     start=False, stop=(kt == CT - 1))
                nc.vector.tensor_copy(
                    v2[nt][:, qtr * 4:(qtr + 1) * 4, 0:D],
                    pv[:].rearrange("p (h d) -> p h d", d=D))
        for nt in range(NT):
            nc.vector.tensor_copy(
                v2[nt][:, :, D:HD],
                ones16.rearrange("p (h one) -> p h one", one=1))

        # prefetch the first half of Wo now (gpsimd cast-DMA f32->bf16) so
        # it is resident long before the output projection needs it
        wo_ts = {}
        for qtr in range(2):
            wo_ts[qtr] = whp.tile([128, CT, 256], bf16, tag="whalf",
                                  name=f"wo{qtr}")
            nc.gpsimd.dma_start(
                out=wo_ts[qtr],
                in_=wo_d.ap().rearrange("(kt p) c -> p kt c", p=128)
                [:, :, qtr * 256:(qtr + 1) * 256])

        # ---------------- phase b: per channel-tile: q/k proj + attention ----
        aoT = []
        for ct in range(CT):
            aoT.append(aop.tile([128, N], bf16, tag="aoT", name=f"aoT{ct}"))

        def recip_normalize(ct, hh, ao65s):
            # denominator row (row 64 of each ao65 half) -> psum COLUMNS via
            # eight K=1 matmuls so the reciprocal runs partition-parallel;
            # then one store + one partition-broadcast DMA (SP queue, kept
            # clear of the bulky weight loads which use the ACT queue).
            dcol = projps.tile([128, 8], f32, tag="proj", name=f"dc{ct}_{hh}")
            for e in range(8):
                nc.tensor.matmul(
                    dcol[:, e:e + 1],
                    ao65s[e // 4][64:65, (e % 4) * 128:(e % 4 + 1) * 128],
                    ones16[64:65, 0:1],
                    start=True, stop=True)
            rcol = rcolp.tile([128, 8], f32, tag="rcol", name=f"rc{ct}_{hh}")
            nc.vector.reciprocal(rcol[:], dcol[:])
            r_dram = rdp.tile([1, N], f32, tag="rdram", name=f"rd{ct}_{hh}")
            nc.sync.dma_start(
                out=r_dram[0, :].rearrange("(e p) -> p e", p=128),
                in_=rcol[:])
            r_bc = rbp.tile([64, N], f32, tag="rbc")
            nc.sync.dma_start(out=r_bc[:],
                              in_=r_dram[0:1, :].partition_broadcast(64))
            for half in range(2):
                nc.vector.tensor_mul(
                    aoT[ct][hh * 64:hh * 64 + 64,
                            half * 512:(half + 1) * 512],
                    ao65s[half][0:64, :],
                    r_bc[:, half * 512:(half + 1) * 512])

        def qk_dma(q):
            # one quarter (256 cols = 2 channel tiles) of Wq/Wk, 1KB bursts
            wq_t = wqkp.tile([128, CT, 256], f32r, tag="wqk", name=f"wq{q}")
            nc.scalar.dma_start(
                out=wq_t,
                in_=wq_d.ap().bitcast(f32r).rearrange("(kt p) c -> p kt c", p=128)
                [:, :, q * 256:(q + 1) * 256])
            wk_t = wqkp.tile([128, CT, 256], f32r, tag="wqk", name=f"wk{q}")
            nc.scalar.dma_start(
                out=wk_t,
                in_=wk_d.ap().bitcast(f32r).rearrange("(kt p) c -> p kt c", p=128)
                [:, :, q * 256:(q + 1) * 256])
            return wq_t, wk_t

        def qk_proj_ops(ct, wq_t, wk_t):
            """Return (qT, kT, ops): ops are deferred closures, executed in
            order, that emit the projection matmuls + copies one at a time so
            they can be interleaved into the scores/exp loop of the previous
            channel tile (keeps the PE busy while ScalarE runs exp)."""
            qT = qkp.tile([128, N], bf16, tag="qkT", name=f"qT{ct}")
            kT = qkp.tile([128, N], bf16, tag="qkT", name=f"kT{ct}")
            ops = []
            state = {}
            for half in range(2):
                for w_t, b_col, dst in ((wq_t, bq_t, qT), (wk_t, bk_t, kT)):
                    def mk_group(half=half, w_t=w_t, b_col=b_col, dst=dst):
                        def alloc():
                            state[(id(w_t), half)] = projps.tile(
                                [128, 512], f32, tag="proj", name="pqk")
                        return alloc
                    alloc = mk_group()
                    c0 = (ct % 2) * 128
                    for kt in range(CT):
                        def mm(kt=kt, half=half, w_t=w_t, alloc=alloc, c0=c0):
                            if kt == 0:
                                alloc()
                            p = state[(id(w_t), half)]
                            nc.tensor.matmul(
                                p[:], w_t[:, kt, c0:c0 + 128],
                                xT[kt][:, half * 512:(half + 1) * 512],
                                start=(kt == 0), stop=(kt == CT - 1))
                        ops.append(mm)
                    def cp(half=half, w_t=w_t, b_col=b_col, dst=dst):
                        p = state[(id(w_t), half)]
                        nc.vector.tensor_scalar_add(
                            dst[:, half * 512:(half + 1) * 512], p[:],
                            b_col[:, ct:ct + 1])
                    ops.append(cp)
            return qT, kT, ops

        wq_quarters = {0: qk_dma(0)}
        qT0, kT0, ops0 = qk_proj_ops(0, *wq_quarters[0])
        for op in ops0:
            op()
        qk_cur = (qT0, kT0)
        next_ops = []
        for ct in range(CT):
            qT, kT = qk_cur
            # prefetch the weight quarter two channel-tiles ahead
            nq = (ct + 2) // 2
            if ct % 2 == 0 and ct + 2 < CT and nq not in wq_quarters:
                wq_quarters[nq] = qk_dma(nq)
            if ct + 1 < CT:
                qTn, kTn, next_ops = qk_proj_ops(ct + 1,
                                                 *wq_quarters[(ct + 1) // 2])
            else:
                qTn = kTn = None
                next_ops = []
            # scores + exp for the 2 heads of this ct, st-wise; AV half-0
            # accumulation chunks trail the exp by one seq tile so the PE
            # never waits on ScalarE.
            pts = []
            for hh in range(2):
                pt = ptp.tile([128, NT, N], bf16, tag="pT", name=f"pT{ct}_{hh}")
                pts.append(pt)
            av0 = []
            for hh in range(2):
                av0.append(avps.tile([65, 512], f32, tag="av",
                                     name=f"av0_{ct}_{hh}"))

            def av0_chunk(st):
                for hh in range(2):
                    nc.tensor.matmul(
                        av0[hh][:],
                        v2[st][:, 2 * ct + hh, :],
                        pts[hh][:, st, 0:512],
                        start=(st == 0), stop=(st == NT - 1))

            for st in range(NT):
                for hh in range(2):
                    r0, r1 = hh * 64, hh * 64 + 64
                    ps = spool.tile([128, N], f32, tag="scores")
                    for half in range(2):
                        nc.tensor.matmul(
                            ps[:, half * 512:(half + 1) * 512],
                            kT[r0:r1, st * 128:(st + 1) * 128],
                            qT[r0:r1, half * 512:(half + 1) * 512],
                            start=True, stop=True)
                    nc.scalar.activation(out=pts[hh][:, st, :], in_=ps[:],
                                         func=mybir.ActivationFunctionType.Exp,
                                         bias=mb[:, st:st + 1], scale=SCALE)
                if st > 1:
                    av0_chunk(st - 2)   # 2 tiles behind: exp surely drained
                # interleave ~5 of the next ct's projection ops to keep the
                # PE fed while ScalarE churns through the exps
                for _ in range(6):
                    if next_ops:
                        next_ops.pop(0)()
            av0_chunk(NT - 2)
            while next_ops:
                next_ops.pop(0)()
            av0_chunk(NT - 1)
            if ct + 1 < CT:
                qk_cur = (qTn, kTn)

            ao65s = {}
            for hh in range(2):
                t = aop65.tile([65, 512], f32, tag="ao65",
                               name=f"ao65_{ct}_{hh}_0")
                nc.vector.tensor_copy(t[:], av0[hh][:])   # frees the bank
                ao65s[hh] = [t]
            # AV half-1: contiguous PE block (exp for this ct already done)
            for hh in range(2):
                av1 = avps.tile([65, 512], f32, tag="av",
                                name=f"av1_{ct}_{hh}")
                for st in range(NT):
                    nc.tensor.matmul(
                        av1[:],
                        v2[st][:, 2 * ct + hh, :],
                        pts[hh][:, st, 512:1024],
                        start=(st == 0), stop=(st == NT - 1))
                t = aop65.tile([65, 512], f32, tag="ao65",
                               name=f"ao65_{ct}_{hh}_1")
                nc.vector.tensor_copy(t[:], av1[:])
                ao65s[hh].append(t)
            for hh in range(2):
                recip_normalize(ct, hh, ao65s[hh])

        # ---------------- phase c: output projection ----------------
        for qtr in range(4):
            if qtr in wo_ts:
                wo_t = wo_ts[qtr]
            else:
                wo_t = whp.tile([128, CT, 256], bf16, tag="whalf",
                                name=f"wo{qtr}")
                nc.gpsimd.dma_start(
                    out=wo_t,
                    in_=wo_d.ap().rearrange(
                        "(kt p) c -> p kt c", p=128)
                    [:, :, qtr * 256:(qtr + 1) * 256])
            for nt in range(NT):
                py = projps.tile([128, 256], f32, tag="proj")
                nc.tensor.matmul(py[:], ones_bf[:],
                                 bo_bf[:, qtr * 256:(qtr + 1) * 256],
                                 start=True, stop=False)
                for kt in range(CT):
                    nc.tensor.matmul(py[:], aoT[kt][:, nt * 128:(nt + 1) * 128],
                                     wo_t[:, kt, :],
                                     start=False, stop=(kt == CT - 1))
                y = yp.tile([128, 256], f32, tag="ysb")
                nc.vector.tensor_copy(y[:], py[:])
                nc.sync.dma_start(
                    out=out_d.ap()[nt * 128:(nt + 1) * 128,
                                   qtr * 256:(qtr + 1) * 256],
                    in_=y[:])

    nc.compile()
    return nc


_NC = None


def _get_nc():
    global _NC
    if _NC is None:
        _NC = _build()
    return _NC


def _in_maps(inputs):
    q = np.ascontiguousarray(np.asarray(inputs["query"], dtype=np.float32))
    mask = np.ascontiguousarray(np.asarray(inputs["mask"], dtype=np.int32))
    shared = {
        "Wq": np.ascontiguousarray(np.asarray(inputs["Wq"], dtype=np.float32)),
        "Wk": np.ascontiguousarray(np.asarray(inputs["Wk"], dtype=np.float32)),
        "Wv": np.ascontiguousarray(np.asarray(inputs["Wv"], dtype=np.float32)),
        "Wo": np.ascontiguousarray(np.asarray(inputs["Wo"], dtype=np.float32)),
        "bq": np.ascontiguousarray(np.asarray(inputs["bq"], dtype=np.float32)),
        "bk": np.ascontiguousarray(np.asarray(inputs["bk"], dtype=np.float32)),
        "bv": np.ascontiguousarray(np.asarray(inputs["bv"], dtype=np.float32)),
        "bo": np.ascontiguousarray(np.asarray(inputs["bo"], dtype=np.float32)),
    }
    in_maps = []
    for b in range(B):
        m = {"x": np.ascontiguousarray(q[b]), "mask": np.ascontiguousarray(mask[b])}
        m.update(shared)
        in_maps.append(m)
    return in_maps


def kernel(**inputs):
    nc = _get_nc()
    res = bass_utils.run_bass_kernel_spmd(nc, _in_maps(inputs),
                                          core_ids=list(range(B)))
    return np.stack([r["out"] for r in res.results]).astype(np.float32)


if __name__ == "__main__":
    rng = np.random.default_rng(0)
    inputs = {
        "query": rng.standard_normal((B, N, C), dtype=np.float32),
        "mask": rng.integers(0, 2, (B, N)).astype(np.int32),
        "Wq": (rng.standard_normal((C, C), dtype=np.float32) * C ** -0.5),
        "bq": np.zeros(C, np.float32),
        "Wk": (rng.standard_normal((C, C), dtype=np.float32) * C ** -0.5),
        "bk": np.zeros(C, np.float32),
        "Wv": (rng.standard_normal((C, C), dtype=np.float32) * C ** -0.5),
        "bv": np.zeros(C, np.float32),
        "Wo": (rng.standard_normal((C, C), dtype=np.float32) * C ** -0.5),
        "bo": np.zeros(C, np.float32),
    }
    out = kernel(**inputs)
    # numpy reference
    def ref(q, mask, Wq, bq, Wk, bk, Wv, bv, Wo, bo):
        Bq, Nq, Cq = q.shape
        qq = (q @ Wq + bq).reshape(Bq, Nq, H, D).transpose(0, 2, 1, 3)
        kk = (q @ Wk + bk).reshape(Bq, Nq, H, D).transpose(0, 2, 1, 3)
        vv = (q @ Wv + bv).reshape(Bq, Nq, H, D).transpose(0, 2, 1, 3)
        at = np.einsum("bhnd,bhsd->bhns", qq, kk) * SCALE
        at = np.where(mask[:, None, None, :] == 0, -np.inf, at)
        at = at - at.max(-1, keepdims=True)
        e = np.exp(at)
        p = e / e.sum(-1, keepdims=True)
        o = np.einsum("bhns,bhsd->bhnd", p, vv)
        o = o.transpose(0, 2, 1, 3).reshape(Bq, Nq, Cq)
        return o @ Wo + bo
    expected = ref(**{k: inputs[k] for k in
                      ["query", "mask", "Wq", "bq", "Wk", "bk", "Wv", "bv", "Wo", "bo"]}
                   .values()) if False else None
    expected = ref(inputs["query"], inputs["mask"], inputs["Wq"], inputs["bq"],
                   inputs["Wk"], inputs["bk"], inputs["Wv"], inputs["bv"],
                   inputs["Wo"], inputs["bo"])
    err = np.abs(out - expected).max() / np.abs(expected).max()
    print("self-test rel err:", err)
